# revision 48
# baseline (speedup 1.0000x reference)
"""Multi-head attention (b=4, n=2048, d=1024, h=16, dh=64) on 8 TRN2 NeuronCores.

Sharding: batch x head-half per core (core c handles batch b=c//2 and the 8
heads h in [ (c%2)*8, (c%2)*8+8 )).  Each core projects Q/K/V only for its
own 512 head-dims (no duplicated projection work), runs flash-style
attention for all 2048 query rows x 2048 keys over its 8 heads, and forms
the PARTIAL output projection out_partial[2048, 1024] = ctx @ WoT[my 512
head-dims, :] (+ bo/2).  A pairwise ReduceScatter (replica groups
[[0,1],[2,3],[4,5],[6,7]]) sums the two partials of each batch and hands
every core the 1024 output rows it owns.  The 2048 query rows are processed
in 4 groups of 512 (256 rows from each output half) so four smaller
ReduceScatters pipeline behind the attention instead of one big one at the
end.

DMA routing: input loads are issued from SP, the ctx transposes and partial
stores from DVE, and the collective-dependent ccout->out copies from Pool
(SWDGE) -- a DMA's dependency wait blocks its issuing engine's sequencer, so
the copies that wait ~28us on a ReduceScatter must not share a queue with
the next group's transposes/stores.

Host-side layout (free transposes in numpy): xt=x[b].T, wqt/wkt/wvt =
W.T[:, my-head-cols], wot = Wo.T[my-head-rows, :], bo2 = bo/2.
"""

import sys

sys.path.insert(0, "/opt/trn_rl_repo")

from contextlib import ExitStack

import numpy as np

import concourse.bass as bass
import concourse.tile as tile
from concourse import bacc, mybir
from concourse.bass_utils import run_bass_kernel_spmd

F32 = mybir.dt.float32
BF16 = mybir.dt.bfloat16
EXP = mybir.ActivationFunctionType.Exp

P = 128
D = 1024  # model dim
N = 2048  # sequence rows per batch (all handled by each core of the pair)
HH = 8  # heads per core
DH = 64  # head dim
DHH = HH * DH  # 512 head-dims per core
SCALE = DH**-0.5  # 0.125
NCORES = 8
NCC = D // P  # 8 contraction chunks over d
NT = DHH // P  # 4 head-pair tiles (2 heads each)
NJB = N // P  # 16 key blocks
NG = 4  # query-row groups
GW = N // NG  # 512 query rows per group (256 from each output half)
GROUPS = [[0, 1], [2, 3], [4, 5], [6, 7]]


def _build():
    nc = bacc.Bacc("TRN2", target_bir_lowering=False, debug=False, num_devices=NCORES)

    xt = nc.dram_tensor("xt", [D, N], BF16, kind="ExternalInput").ap()
    wqt = nc.dram_tensor("wqt", [D, DHH], BF16, kind="ExternalInput").ap()
    wkt = nc.dram_tensor("wkt", [D, DHH], BF16, kind="ExternalInput").ap()
    wvt = nc.dram_tensor("wvt", [D, DHH], BF16, kind="ExternalInput").ap()
    wot = nc.dram_tensor("wot", [DHH, D], BF16, kind="ExternalInput").ap()
    bo2 = nc.dram_tensor("bo2", [1, D], F32, kind="ExternalInput").ap()
    id128 = nc.dram_tensor("id128", [P, P], BF16, kind="ExternalInput").ap()
    out = nc.dram_tensor("out", [N // 2, D], BF16, kind="ExternalOutput").ap()
    ccin = [
        nc.dram_tensor(f"ccin{g}", [GW, D], BF16, kind="Internal").ap()
        for g in range(NG)
    ]
    ccout = nc.dram_tensor("ccout", [N // 2, D], BF16, kind="Internal").ap()

    with tile.TileContext(nc) as tc, ExitStack() as octx:
        # PSUM: scores 2x[128,1024] (4 banks) + AV 2x[128,65] (2) + proj 2x[128,512] (2)
        psS = octx.enter_context(tc.tile_pool(name="psS", bufs=2, space="PSUM"))
        psAV = octx.enter_context(tc.tile_pool(name="psAV", bufs=2, space="PSUM"))
        psP = octx.enter_context(tc.tile_pool(name="psP", bufs=2, space="PSUM"))


        # resident inputs / weights
        xp = octx.enter_context(tc.tile_pool(name="xp", bufs=1))
        wkp = octx.enter_context(tc.tile_pool(name="wkp", bufs=1))
        wqp = octx.enter_context(tc.tile_pool(name="wqp", bufs=1))
        wvp = octx.enter_context(tc.tile_pool(name="wvp", bufs=1))
        wop = octx.enter_context(tc.tile_pool(name="wop", bufs=1))
        bip = octx.enter_context(tc.tile_pool(name="bip", bufs=1))
        XTA = xp.tile([P, NCC, N], BF16, name="xta")
        WKT = [wkp.tile([P, NCC, P], BF16, tag=f"wk{t}", name=f"wk{t}") for t in range(NT)]
        WQT = [wqp.tile([P, NCC, P], BF16, tag=f"wq{t}", name=f"wq{t}") for t in range(NT)]
        WVT = [wvp.tile([P, NCC, P], BF16, tag=f"wv{t}", name=f"wv{t}") for t in range(NT)]
        WOT = [wop.tile([P, D], BF16, tag=f"wo{f}", name=f"wo{f}") for f in range(NT)]
        BIAS = bip.tile([P, D], F32, name="BIAS")
        ID128 = bip.tile([P, P], BF16, name="ID128")

        # attention state
        ktp = octx.enter_context(tc.tile_pool(name="ktp", bufs=1, side="right"))
        qtp = octx.enter_context(tc.tile_pool(name="qtp", bufs=1, side="right"))
        vp = octx.enter_context(tc.tile_pool(name="vp", bufs=1, side="right"))
        KT = [ktp.tile([P, N], BF16, tag=f"kt{t}", name=f"kt{t}") for t in range(NT)]
        QT = [qtp.tile([P, N], BF16, tag=f"qt{t}", name=f"qt{t}") for t in range(NT)]
        vall = vp.tile([P, NJB, HH, DH + 1], BF16, tag="vall", name="vall")

        esp = octx.enter_context(tc.tile_pool(name="esp", bufs=11))
        recp = octx.enter_context(tc.tile_pool(name="recp", bufs=6))
        stp = octx.enter_context(tc.tile_pool(name="stp", bufs=3))
        ctxp = octx.enter_context(tc.tile_pool(name="ctxp", bufs=1, side="right"))
        CTXT = [
            [
                ctxp.tile([P, GW], BF16, tag=f"ctx{t}_{gb}", name=f"ctx{t}_{gb}")
                for gb in range(2)
            ]
            for t in range(NT)
        ]
        pstp = octx.enter_context(tc.tile_pool(name="pstp", bufs=4))
        # f32 accumulators for the incremental partial-Wo: one [128, 512] block
        # per (q, e), double-buffered across groups
        accp = octx.enter_context(tc.tile_pool(name="accp", bufs=2))

        # ---------------- prefetch (SP queue) ----------------
        # column-block order 0,2,1,3: group 0's query rows are cols [0:256] and
        # [1024:1280], so cb2 must land right after cb0.
        def load_w(dst, src, t):
            nc.sync.dma_start(
                dst[:],
                src[:, t * P : (t + 1) * P].rearrange("(c p) o -> p c o", p=P),
            )

        def load_x(c0, c1):
            nc.sync.dma_start(
                XTA[:, :, c0:c1],
                xt[:, c0:c1].rearrange("(c p) n -> p c n", p=P),
            )

        # group 0's query slab (cols 1024:1280) right after key block 0 so
        # proj_q(0, 0) and the first scores can start ~8us in
        load_w(WKT[0], wkt, 0)
        load_x(0, 512)
        load_w(WQT[0], wqt, 0)
        load_x(1024, 1280)
        load_x(1280, 1536)
        load_x(512, 1024)
        load_x(1536, 2048)
        load_w(WVT[0], wvt, 0)
        for t in range(1, NT):
            load_w(WKT[t], wkt, t)
            load_w(WQT[t], wqt, t)
            load_w(WVT[t], wvt, t)
        for f in range(NT):
            nc.sync.dma_start(WOT[f][:], wot[f * P : (f + 1) * P, :])
        nc.gpsimd.dma_start(BIAS[:], bo2.to_broadcast([P, D]))
        nc.gpsimd.dma_start(ID128[:], id128)
        nc.vector.memset(vall[:, :, :, DH : DH + 1], 1.0)

        def proj_k_nb(t, nb):
            """One 512-key block of KT[t]; nb order follows x col-block arrival."""
            ps = psP.tile([P, 512], F32, tag="pj", name="pj")
            for c in range(NCC):
                nc.tensor.matmul(
                    ps[:],
                    WKT[t][:, c, :],
                    XTA[:, c, nb * 512 : (nb + 1) * 512],
                    start=(c == 0),
                    stop=(c == NCC - 1),
                )
            nc.vector.tensor_copy(KT[t][:, nb * 512 : (nb + 1) * 512], ps[:])

        def proj_v_jq(t, jq):
            """V for head pair t, key blocks jq*4..jq*4+3.  Stationary is the x
            chunk so psum partitions are the keys of block j; the psum
            [128, 512] packs 4 j-blocks' [keys, 2*64] slabs."""
            ps = psP.tile([P, 512], F32, tag="pj", name="pj")
            for j4 in range(4):
                j = jq * 4 + j4
                for c in range(NCC):
                    nc.tensor.matmul(
                        ps[:, j4 * P : (j4 + 1) * P],
                        XTA[:, c, j * P : (j + 1) * P],
                        WVT[t][:, c, :],
                        start=(c == 0),
                        stop=(c == NCC - 1),
                    )
            nc.vector.tensor_copy(
                vall[:, jq * 4 : (jq + 1) * 4, 2 * t : 2 * t + 2, 0:DH],
                ps[:].rearrange("p (j h d) -> p j h d", j=4, h=2),
            )

        def proj_q(t, g):
            """QT[t][:, g*512:(g+1)*512]: group g = global rows [g*256,(g+1)*256)
            of the low output half then of the high output half."""
            ps = psP.tile([P, 512], F32, tag="pj", name="pj")
            for sl in range(2):
                src = sl * (N // 2) + g * 256
                for c in range(NCC):
                    nc.tensor.matmul(
                        ps[:, sl * 256 : (sl + 1) * 256],
                        WQT[t][:, c, :],
                        XTA[:, c, src : src + 256],
                        start=(c == 0),
                        stop=(c == NCC - 1),
                    )
            nc.vector.tensor_copy(QT[t][:, g * 512 : (g + 1) * 512], ps[:])

        # ---------------- main loop: groups of 512 query rows ----------------
        def attention(t, g, fillers=(), prs=None):
            """Both heads of pair t over the group's 512 query cols; `fillers`
            are projection emitters woven between the score tiles of head 0 so
            the tensor engine has group-0 projection work while ACT chews
            exps.  `prs` permutes the j-pair emission order (data arrival)."""
            gslc = slice(g * GW, (g + 1) * GW)
            fillers = list(fillers)
            prs = list(prs) if prs is not None else list(range(NJB // 2))
            stg = stp.tile([P, 4 * 2 * DH], BF16, tag="st", name="st")
            for hh in range(2):
                h = 2 * t + hh
                dp = hh * DH
                es_list = [None] * (NJB // 2)
                for pr in prs:
                    sp = psS.tile([P, 1024], F32, tag="sp", name="sp")
                    for half in range(2):
                        j = pr * 2 + half
                        nc.tensor.matmul(
                            sp[:, half * 512 : (half + 1) * 512],
                            KT[t][dp : dp + DH, j * P : (j + 1) * P],
                            QT[t][dp : dp + DH, gslc],
                            start=True,
                            stop=True,
                        )
                    es = esp.tile([P, 1024], BF16, tag="es", name="es")
                    nc.scalar.activation(es[:], sp[:], EXP, scale=SCALE)
                    es_list[pr] = es
                    if fillers:
                        fillers.pop(0)()

                for q in range(4):
                    ctp = psAV.tile([P, DH + 1], F32, tag="ct", name="ct")
                    for j in range(NJB):
                        nc.tensor.matmul(
                            ctp[:],
                            es_list[j // 2][
                                :, (j % 2) * 512 + q * P : (j % 2) * 512 + (q + 1) * P
                            ],
                            vall[:, j, h, :],
                            start=(j == 0),
                            stop=(j == NJB - 1),
                        )
                    rec = recp.tile([P, 1], F32, tag="rec", name="rec")
                    nc.vector.reciprocal(rec[:], ctp[:, DH : DH + 1])
                    nc.vector.tensor_scalar_mul(
                        stg[:, q * P + dp : q * P + dp + DH], ctp[:, 0:DH], rec[:]
                    )
            for f in fillers:
                f()
            return stg

        def transpose_ctx(t, g, stg, q):
            """PE-based 128x128 transpose of [i, dims] -> CTXT[t][dims, i].
            (DmaTransposeAnt is serialized against collectives by the tile
            scheduler, which would stall every transpose behind an in-flight
            ReduceScatter.)"""
            pst = psAV.tile([P, P], BF16, tag="ct", name="tr")
            nc.tensor.transpose(pst[:], stg[:, q * P : (q + 1) * P], ID128[:])
            nc.vector.tensor_copy(CTXT[t][g % 2][:, q * P : (q + 1) * P], pst[:])

        def wo_partial(t, g, accs, pstgs, qs=(0, 1, 2, 3)):
            """Fold pair t's ctx into the group's partial-output accumulators.
            f==0 seeds with bias/2, f==3 emits the bf16 store stage."""
            for q in qs:
                for e in range(2):
                    ps = psP.tile([P, 512], F32, tag="pj", name="pj")
                    nc.tensor.matmul(
                        ps[:],
                        CTXT[t][g % 2][:, q * P : (q + 1) * P],
                        WOT[t][:, e * 512 : (e + 1) * 512],
                        start=True,
                        stop=True,
                    )
                    if t == 0:
                        acc = accp.tile(
                            [P, 512], F32, tag=f"acc{q}_{e}", name=f"acc{q}_{e}"
                        )
                        accs[(q, e)] = acc
                        nc.vector.tensor_add(
                            acc[:], ps[:], BIAS[:, e * 512 : (e + 1) * 512]
                        )
                    elif t < NT - 1:
                        acc = accs[(q, e)]
                        nc.vector.tensor_add(acc[:], ps[:], acc[:])
                    else:
                        if e == 0:
                            pstgs[q] = pstp.tile([P, D], BF16, tag="pst", name="pst")
                        nc.vector.tensor_add(
                            pstgs[q][:, e * 512 : (e + 1) * 512], ps[:], accs[(q, e)][:]
                        )

        def pair_tail(t, g, stg, accs, pstgs):
            """Transposes + partial-Wo + next proj_q for pair (t, g), bundled
            as filler closures woven into the NEXT pair's score stream."""
            fs = [
                lambda q=q: (
                    transpose_ctx(t, g, stg, q),
                    wo_partial(t, g, accs, pstgs, qs=(q,)),
                )
                for q in range(4)
            ]
            if g < NG - 1:
                fs.append(lambda: proj_q(t, g + 1))
            return fs

        tail_fs = []
        state = {}
        for g in range(NG):
            accs, pstgs = {}, {}
            state[g] = (accs, pstgs)
            for t in range(NT):
                fillers, prs = [], None
                if g == 0:
                    # keys 0:512 of this pair before its scores; the rest of
                    # its K blocks and its V slices woven between score tiles.
                    # The pair's own K fillers MUST precede the previous
                    # pair's tail so KT[t] block nb lands before the score
                    # tile that reads it.
                    proj_k_nb(t, 0)
                    if t == 0:
                        proj_q(0, 0)
                        # j-pair emission follows x col-block DMA arrival
                        prs = [0, 1, 4, 5, 2, 3, 6, 7]
                        korder = (2, 1, 3)
                    else:
                        korder = (1, 2, 3)
                    fillers = (
                        [lambda t=t, nb=nb: proj_k_nb(t, nb) for nb in korder]
                        + [lambda t=t, jq=jq: proj_v_jq(t, jq) for jq in range(4)]
                    )
                fillers += list(tail_fs)
                stg = attention(t, g, fillers, prs)
                if g == 0 and t < NT - 1:
                    proj_q(t + 1, 0)
                tail_fs = pair_tail(t, g, stg, accs, pstgs)
            # flush the last pair's tail, then store this group's partials
            for f in tail_fs:
                f()
            tail_fs = []
            for q in range(4):
                nc.sync.dma_start(ccin[g][q * P : (q + 1) * P, :], pstgs[q][:])
            # pairwise ReduceScatter of this group's partial rows; each core
            # receives the 256 rows of its own output half.
            nc.gpsimd.collective_compute(
                "ReduceScatter",
                mybir.AluOpType.add,
                replica_groups=GROUPS,
                ins=[ccin[g]],
                outs=[ccout[g * 256 : (g + 1) * 256, :]],
            )
        # ccout -> out copies on the Pool queue: Pool only runs collectives and
        # these copies, so their long collective-waits cannot block other DMA
        # queues regardless of scheduler placement.  Groups 0-2 copy out while
        # the last ReduceScatter flies; only the 256-row tail copy waits on it.
        nc.gpsimd.dma_start(out[0 : 3 * 256, :], ccout[0 : 3 * 256, :])
        nc.gpsimd.dma_start(out[3 * 256 :, :], ccout[3 * 256 :, :])

    nc.compile()
    return nc


_NC = None


def _get_nc():
    global _NC
    if _NC is None:
        _NC = _build()
    return _NC


def _make_in_maps(x, Wq, Wk, Wv, Wo, bo):
    import ml_dtypes

    bf16 = ml_dtypes.bfloat16
    wqT = np.ascontiguousarray(Wq.T).astype(bf16)
    wkT = np.ascontiguousarray(Wk.T).astype(bf16)
    wvT = np.ascontiguousarray(Wv.T).astype(bf16)
    woT = np.ascontiguousarray(Wo.T).astype(bf16)
    bo2 = np.ascontiguousarray(bo.reshape(1, D)).astype(np.float32) * 0.5
    eye = np.eye(P, dtype=np.float32).astype(bf16)
    xts = [np.ascontiguousarray(x[b].T).astype(bf16) for b in range(x.shape[0])]
    in_maps = []
    for c in range(NCORES):
        b, r = c // 2, c % 2
        hs = slice(r * DHH, (r + 1) * DHH)
        in_maps.append(
            {
                "xt": xts[b],
                "wqt": np.ascontiguousarray(wqT[:, hs]),
                "wkt": np.ascontiguousarray(wkT[:, hs]),
                "wvt": np.ascontiguousarray(wvT[:, hs]),
                "wot": np.ascontiguousarray(woT[hs, :]),
                "bo2": bo2,
                "id128": eye,
            }
        )
    return in_maps


def _run(x, Wq, Wk, Wv, Wo, bo, **spmd_kwargs):
    nc = _get_nc()
    in_maps = _make_in_maps(x, Wq, Wk, Wv, Wo, bo)
    res = run_bass_kernel_spmd(nc, in_maps, list(range(NCORES)), **spmd_kwargs)
    outs = [np.asarray(res.results[c]["out"]) for c in range(NCORES)]
    full = np.concatenate(outs, axis=0).reshape(4, 2048, D).astype(np.float32)
    return full, res


def kernel(x, Wq, Wk, Wv, Wo, bo):
    full, _ = _run(
        np.asarray(x), np.asarray(Wq), np.asarray(Wk), np.asarray(Wv),
        np.asarray(Wo), np.asarray(bo),
    )
    return full


# revision 68
# speedup vs baseline: 1.0499x; 1.0499x over previous
"""Multi-head attention (b=4, n=2048, d=1024, h=16, dh=64) on 8 TRN2 NeuronCores.

Sharding: batch x sequence-half per core (core c handles batch b=c//2, query
rows s=(c%2)*1024 .. +1024). Each core recomputes K/V for its whole batch
locally (no collectives), computes flash-style attention for its 1024 query
rows over all 16 heads, applies the output projection, and writes a disjoint
1024-row slice of the flattened output. Matmuls run in fp32r (TF32-like fast
fp32 mode); the attention AV product runs in bf16.

Host-side layout choices (free transposes/permutes in numpy):
  xtkv [d, 2048] = concat(x[b, my_half].T, x[b, other_half].T) -- the core's
      own query rows are ALWAYS columns 0:1024, so the same SPMD graph works
      on every core, and key order permutation is softmax-invariant.
  wqt/wkt/wvt/wot = W.T (contraction dim first), bo as [1, d].
"""

import sys

sys.path.insert(0, "/opt/trn_rl_repo")

from contextlib import ExitStack

import numpy as np

import concourse.bass as bass
import concourse.tile as tile
from concourse import bacc, mybir
from concourse.bass_utils import run_bass_kernel_spmd

F32 = mybir.dt.float32
F32R = mybir.dt.float32r
BF16 = mybir.dt.bfloat16
EXP = mybir.ActivationFunctionType.Exp

P = 128
D = 1024  # model dim
NI = 1024  # query rows per core
NJ = 2048  # key rows per core (full batch)
H = 16  # heads
DH = 64  # head dim
SCALE = DH**-0.5  # 0.125
NCORES = 8

NCC = D // P  # 8 contraction chunks
NDB = D // P  # 8 feature blocks


def _build():
    nc = bacc.Bacc("TRN2", target_bir_lowering=False, debug=False, num_devices=NCORES)

    xtkv = nc.dram_tensor("xtkv", [D, NJ], BF16, kind="ExternalInput").ap()
    wqt = nc.dram_tensor("wqt", [D, D], BF16, kind="ExternalInput").ap()
    wkt = nc.dram_tensor("wkt", [D, D], BF16, kind="ExternalInput").ap()
    wvt = nc.dram_tensor("wvt", [D, D], BF16, kind="ExternalInput").ap()
    wot = nc.dram_tensor("wot", [D, D], BF16, kind="ExternalInput").ap()
    bo = nc.dram_tensor("bo", [1, D], F32, kind="ExternalInput").ap()
    out = nc.dram_tensor("out", [NI, D], BF16, kind="ExternalOutput").ap()
    DEBUG = bool(__import__("os").environ.get("KERNEL_DEBUG"))
    if DEBUG:
        dbg_kt0 = nc.dram_tensor("dbg_kt0", [P, NJ], BF16, kind="ExternalOutput").ap()
        dbg_qt0 = nc.dram_tensor("dbg_qt0", [P, NI], BF16, kind="ExternalOutput").ap()
        dbg_v0 = nc.dram_tensor("dbg_v0", [P, H, DH + 1], BF16, kind="ExternalOutput").ap()
        dbg_ctx0 = nc.dram_tensor("dbg_ctx0", [P, NI], BF16, kind="ExternalOutput").ap()

    with tile.TileContext(nc) as tc, ExitStack() as octx:
        # kernel-wide PSUM pools: 4 + 2 + 2 = 8 banks
        psA = octx.enter_context(tc.tile_pool(name="psA", bufs=2, space="PSUM"))
        psB = octx.enter_context(tc.tile_pool(name="psB", bufs=2, space="PSUM"))
        psC = octx.enter_context(tc.tile_pool(name="psC", bufs=2, space="PSUM"))
        dramp = octx.enter_context(tc.tile_pool(name="dramp", bufs=4, space="DRAM"))

        kt_pool = octx.enter_context(tc.tile_pool(name="ktp", bufs=1))
        qt_pool = octx.enter_context(tc.tile_pool(name="qtp", bufs=1))
        v_pool = octx.enter_context(tc.tile_pool(name="vp", bufs=1))
        KT = [kt_pool.tile([P, NJ], BF16, tag=f"kt{i}", name=f"kt{i}") for i in range(NDB)]
        QT = [qt_pool.tile([P, NI], BF16, tag=f"qt{i}", name=f"qt{i}") for i in range(NDB)]
        vall = v_pool.tile([P, NJ // P, H, DH + 1], BF16, tag="vall", name="vall")
        V = [vall[:, j] for j in range(NJ // P)]

        # attention pools first: their SBUF must not alias the projection pools
        ctx_pool = octx.enter_context(tc.tile_pool(name="ctxp", bufs=1, side="right"))
        CTX = [ctx_pool.tile([P, NI], BF16, tag=f"ctx{t}", name=f"ctx{t}") for t in range(NDB)]
        esp = octx.enter_context(tc.tile_pool(name="es", bufs=11))
        recp = octx.enter_context(tc.tile_pool(name="rec", bufs=6))
        stp = octx.enter_context(tc.tile_pool(name="stg", bufs=10))

        # ---------------- phase Q (bf16); XQ reused by K; WK/XKB prefetched ----
        xqp = octx.enter_context(tc.tile_pool(name="xq", bufs=1))
        XQ = [xqp.tile([P, NI], BF16, tag=f"xq{c}", name=f"xq{c}") for c in range(NCC)]
        wkp = octx.enter_context(tc.tile_pool(name="wk", bufs=1))
        WK = [wkp.tile([P, D], BF16, tag=f"wk{c}", name=f"wk{c}") for c in range(NCC)]
        with tc.tile_pool(name="wq", bufs=1) as wqp:
            WQ = [wqp.tile([P, D], BF16, tag=f"wq{c}", name=f"wq{c}") for c in range(NCC)]
            for c in range(NCC):
                nc.sync.dma_start(XQ[c][:], xtkv[c * P : (c + 1) * P, 0:NI])
                nc.sync.dma_start(WQ[c][:], wqt[c * P : (c + 1) * P, :])
            for c in range(NCC):
                nc.sync.dma_start(WK[c][:], wkt[c * P : (c + 1) * P, :])
            for db in range(NDB):
                for ib in range(NI // 512):
                    ps = psB.tile([P, 512], F32, tag="pj", name="pj")
                    for c in range(NCC):
                        nc.tensor.matmul(
                            ps[:],
                            WQ[c][:, db * P : (db + 1) * P],
                            XQ[c][:, ib * 512 : (ib + 1) * 512],
                            start=(c == 0),
                            stop=(c == NCC - 1),
                        )
                    nc.vector.tensor_copy(QT[db][:, ib * 512 : (ib + 1) * 512], ps[:])

        # Wo pools open before the fused scope so WO/bias prefetch during it
        wop = octx.enter_context(tc.tile_pool(name="wo", bufs=1))
        bip = octx.enter_context(tc.tile_pool(name="bias", bufs=1))
        osp = octx.enter_context(tc.tile_pool(name="os", bufs=6))
        WO = [wop.tile([P, D], BF16, tag=f"wo{f}", name=f"wo{f}") for f in range(NCC)]
        for f in range(NCC):
            nc.sync.dma_start(WO[f][:], wot[f * P : (f + 1) * P, :])
        BIAS = bip.tile([P, D], F32, name="BIAS")
        nc.gpsimd.dma_start(BIAS[:], bo.to_broadcast([P, D]))

        # -------- fused phase K + attention: per db, project KT[db] then emit
        # the two heads (x2 ib blocks) that consume it ------------------------
        with (
            tc.tile_pool(name="xkb", bufs=1) as xkbp,
            tc.tile_pool(name="wvh", bufs=1) as wvhp,
        ):
            XKB = [xkbp.tile([P, NI], BF16, tag=f"xkb{c}", name=f"xkb{c}") for c in range(NCC)]
            XKA = XQ
            for c in range(NCC):
                nc.sync.dma_start(XKB[c][:], xtkv[c * P : (c + 1) * P, NI:NJ])
            for j in range(NJ // P):
                nc.vector.memset(V[j][:, :, DH : DH + 1], 1.0)

            def v_halfpass_jg(vh, jg):
                """Project V head-half vh for key group jg (4 j-blocks); x is
                sliced straight out of the resident XKA/XKB tiles."""
                for j4 in range(4):
                    j = jg * 4 + j4
                    xh = XKA if j < 8 else XKB
                    jloc = j % 8
                    ps = psB.tile([P, 512], F32, tag="pj", name="pj")
                    for c in range(NCC):
                        nc.tensor.matmul(
                            ps[:],
                            xh[c][:, jloc * P : (jloc + 1) * P],
                            WVH[c][:],
                            start=(c == 0),
                            stop=(c == NCC - 1),
                        )
                    nc.vector.tensor_copy(
                        V[j][:, vh * 8 : (vh + 1) * 8, 0:DH],
                        ps[:].rearrange("p (h d) -> p h d", h=8),
                    )

            WVH = [wvhp.tile([P, 512], BF16, tag=f"wvh{c}", name=f"wvh{c}") for c in range(NCC)]
            # V head-half 0 (heads 0-7): needed from db=0
            for c in range(NCC):
                nc.sync.dma_start(WVH[c][:], wvt[c * P : (c + 1) * P, 0:512])
            for jg in range(NJ // 512):
                v_halfpass_jg(0, jg)

            for db in range(NDB):
                # V head-half 1 (heads 8-15): one key group per db in 1..4
                if db == 1:
                    WVH = [
                        wvhp.tile([P, 512], BF16, tag=f"wvh{c}", name=f"wvh{c}2")
                        for c in range(NCC)
                    ]
                    for c in range(NCC):
                        nc.sync.dma_start(WVH[c][:], wvt[c * P : (c + 1) * P, 512:1024])
                if 1 <= db <= 4:
                    v_halfpass_jg(1, db - 1)
                # K projection for this db
                for jb in range(NJ // 512):
                    half = XKA if jb < 2 else XKB
                    cslc = slice((jb % 2) * 512, (jb % 2) * 512 + 512)
                    ps = psB.tile([P, 512], F32, tag="pj", name="pj")
                    for c in range(NCC):
                        nc.tensor.matmul(
                            ps[:],
                            WK[c][:, db * P : (db + 1) * P],
                            half[c][:, cslc],
                            start=(c == 0),
                            stop=(c == NCC - 1),
                        )
                    nc.vector.tensor_copy(KT[db][:, jb * 512 : (jb + 1) * 512], ps[:])
                # the two heads living in KT[db], for both i-blocks
                t = db
                for ib in range(NI // 512):
                    islc = slice(ib * 512, (ib + 1) * 512)
                    stgs = {}
                    for hh in range(2):
                        h = 2 * db + hh
                        dp = hh * DH
                        es_list = []
                        for pr in range(NJ // 256):
                            sp = psA.tile([P, 1024], F32, tag="sp", name="sp")
                            for half2 in range(2):
                                j = pr * 2 + half2
                                nc.tensor.matmul(
                                    sp[:, half2 * 512 : (half2 + 1) * 512],
                                    KT[t][dp : dp + DH, j * P : (j + 1) * P],
                                    QT[t][dp : dp + DH, islc],
                                    start=True,
                                    stop=True,
                                )
                            es = esp.tile([P, 1024], BF16, tag="es", name="es")
                            nc.scalar.activation(es[:], sp[:], EXP, scale=SCALE)
                            es_list.append(es)
                        # AV with es STATIONARY and V|1 MOVING: N=65 cycles per
                        # matmul instead of 512 -- output [i, d|sum] per i-128
                        # block, so the softmax divide is a native per-partition
                        # tensor_scalar, and a bf16 DMA-transpose restores the
                        # [f, i] layout Wo consumes.
                        for q in range(4):
                            ctp = psC.tile([P, DH + 1], F32, tag="ct", name="ct")
                            for j in range(NJ // P):
                                nc.tensor.matmul(
                                    ctp[:],
                                    es_list[j // 2][
                                        :,
                                        (j % 2) * 512 + q * P : (j % 2) * 512 + (q + 1) * P,
                                    ],
                                    V[j][:, h, :],
                                    start=(j == 0),
                                    stop=(j == NJ // P - 1),
                                )
                            rec = recp.tile([P, 1], F32, tag="rec", name="rec")
                            nc.vector.reciprocal(rec[:], ctp[:, DH : DH + 1])
                            if hh == 0:
                                stgs[q] = stp.tile([P, 2 * DH], BF16, tag="st", name="st")
                            stg = stgs[q]
                            nc.vector.tensor_scalar_mul(
                                stg[:, dp : dp + DH], ctp[:, 0:DH], rec[:]
                            )
                            if hh == 1:
                                # both heads of the pair staged: one 128-wide
                                # bf16 DMA-transpose fills CTX tile t's columns
                                nc.sync.dma_start_transpose(
                                    CTX[t][:, ib * 512 + q * P : ib * 512 + (q + 1) * P],
                                    stg[:],
                                )

        if DEBUG:
            nc.sync.dma_start(dbg_ctx0, CTX[0][:])

        # ---------------- phase Wo: out = CTX.T @ WoT + bo ----------------------
        if True:
            for ib8 in range(NI // P):
                for eb in range(2):
                    ps = psB.tile([P, 512], F32, tag="pj", name="pj")
                    for f in range(NCC):
                        nc.tensor.matmul(
                            ps[:],
                            CTX[f][:, ib8 * P : (ib8 + 1) * P],
                            WO[f][:, eb * 512 : (eb + 1) * 512],
                            start=(f == 0),
                            stop=(f == NCC - 1),
                        )
                    ostage = osp.tile([P, 512], BF16, tag="os", name="os")
                    nc.vector.tensor_add(
                        ostage[:], ps[:], BIAS[:, eb * 512 : (eb + 1) * 512]
                    )
                    nc.sync.dma_start(
                        out[ib8 * P : (ib8 + 1) * P, eb * 512 : (eb + 1) * 512],
                        ostage[:],
                    )

    nc.compile()
    return nc


_NC = None


def _get_nc():
    global _NC
    if _NC is None:
        _NC = _build()
    return _NC


def _make_in_maps(x, Wq, Wk, Wv, Wo, bo):
    import ml_dtypes

    bf16 = ml_dtypes.bfloat16
    wqt = np.ascontiguousarray(Wq.T).astype(bf16)
    wkt = np.ascontiguousarray(Wk.T).astype(bf16)
    wvt = np.ascontiguousarray(Wv.T).astype(bf16)
    wot = np.ascontiguousarray(Wo.T).astype(bf16)
    bo2 = np.ascontiguousarray(bo.reshape(1, D)).astype(np.float32)
    in_maps = []
    for c in range(NCORES):
        b, s = c // 2, c % 2
        mine = x[b, s * NI : (s + 1) * NI, :].T
        other = x[b, (1 - s) * NI : (2 - s) * NI, :].T
        xtkv = np.ascontiguousarray(np.concatenate([mine, other], axis=1)).astype(bf16)
        in_maps.append(
            {"xtkv": xtkv, "wqt": wqt, "wkt": wkt, "wvt": wvt, "wot": wot, "bo": bo2}
        )
    return in_maps


def _run(x, Wq, Wk, Wv, Wo, bo, **spmd_kwargs):
    nc = _get_nc()
    in_maps = _make_in_maps(x, Wq, Wk, Wv, Wo, bo)
    res = run_bass_kernel_spmd(nc, in_maps, list(range(NCORES)), **spmd_kwargs)
    outs = [np.asarray(res.results[c]["out"]) for c in range(NCORES)]
    full = np.concatenate(outs, axis=0).reshape(4, 2048, D).astype(np.float32)
    return full, res


def kernel(x, Wq, Wk, Wv, Wo, bo):
    full, _ = _run(
        np.asarray(x), np.asarray(Wq), np.asarray(Wk), np.asarray(Wv),
        np.asarray(Wo), np.asarray(bo),
    )
    return full



# revision 71
# speedup vs baseline: 1.0607x; 1.0103x over previous
"""Multi-head attention (b=4, n=2048, d=1024, h=16, dh=64) on 8 TRN2 NeuronCores.

Sharding: batch x sequence-half per core (core c handles batch b=c//2, query
rows s=(c%2)*1024 .. +1024). Each core recomputes K/V for its whole batch
locally (no collectives), computes flash-style attention for its 1024 query
rows over all 16 heads, applies the output projection, and writes a disjoint
1024-row slice of the flattened output. Matmuls run in fp32r (TF32-like fast
fp32 mode); the attention AV product runs in bf16.

Host-side layout choices (free transposes/permutes in numpy):
  xtkv [d, 2048] = concat(x[b, my_half].T, x[b, other_half].T) -- the core's
      own query rows are ALWAYS columns 0:1024, so the same SPMD graph works
      on every core, and key order permutation is softmax-invariant.
  wqt/wkt/wvt/wot = W.T (contraction dim first), bo as [1, d].
"""

import sys

sys.path.insert(0, "/opt/trn_rl_repo")

from contextlib import ExitStack

import numpy as np

import concourse.bass as bass
import concourse.tile as tile
from concourse import bacc, mybir
from concourse.bass_utils import run_bass_kernel_spmd

F32 = mybir.dt.float32
F32R = mybir.dt.float32r
BF16 = mybir.dt.bfloat16
EXP = mybir.ActivationFunctionType.Exp

P = 128
D = 1024  # model dim
NI = 1024  # query rows per core
NJ = 2048  # key rows per core (full batch)
H = 16  # heads
DH = 64  # head dim
SCALE = DH**-0.5  # 0.125
NCORES = 8

NCC = D // P  # 8 contraction chunks
NDB = D // P  # 8 feature blocks


def _build():
    nc = bacc.Bacc("TRN2", target_bir_lowering=False, debug=False, num_devices=NCORES)

    xtkv = nc.dram_tensor("xtkv", [D, NJ], BF16, kind="ExternalInput").ap()
    wqt = nc.dram_tensor("wqt", [D, D], BF16, kind="ExternalInput").ap()
    wkt = nc.dram_tensor("wkt", [D, D], BF16, kind="ExternalInput").ap()
    wvt = nc.dram_tensor("wvt", [D, D], BF16, kind="ExternalInput").ap()
    wot = nc.dram_tensor("wot", [D, D], BF16, kind="ExternalInput").ap()
    bo = nc.dram_tensor("bo", [1, D], F32, kind="ExternalInput").ap()
    out = nc.dram_tensor("out", [NI, D], BF16, kind="ExternalOutput").ap()
    DEBUG = bool(__import__("os").environ.get("KERNEL_DEBUG"))
    if DEBUG:
        dbg_kt0 = nc.dram_tensor("dbg_kt0", [P, NJ], BF16, kind="ExternalOutput").ap()
        dbg_qt0 = nc.dram_tensor("dbg_qt0", [P, NI], BF16, kind="ExternalOutput").ap()
        dbg_v0 = nc.dram_tensor("dbg_v0", [P, H, DH + 1], BF16, kind="ExternalOutput").ap()
        dbg_ctx0 = nc.dram_tensor("dbg_ctx0", [P, NI], BF16, kind="ExternalOutput").ap()

    with tile.TileContext(nc) as tc, ExitStack() as octx:
        # kernel-wide PSUM pools: 4 + 2 + 2 = 8 banks
        psA = octx.enter_context(tc.tile_pool(name="psA", bufs=2, space="PSUM"))
        psB = octx.enter_context(tc.tile_pool(name="psB", bufs=2, space="PSUM"))
        psC = octx.enter_context(tc.tile_pool(name="psC", bufs=2, space="PSUM"))
        dramp = octx.enter_context(tc.tile_pool(name="dramp", bufs=4, space="DRAM"))

        kt_pool = octx.enter_context(tc.tile_pool(name="ktp", bufs=1))
        qt_pool = octx.enter_context(tc.tile_pool(name="qtp", bufs=1))
        v_pool = octx.enter_context(tc.tile_pool(name="vp", bufs=1))
        KT = [kt_pool.tile([P, NJ], BF16, tag=f"kt{i}", name=f"kt{i}") for i in range(NDB)]
        QT = [qt_pool.tile([P, NI], BF16, tag=f"qt{i}", name=f"qt{i}") for i in range(NDB)]
        vall = v_pool.tile([P, NJ // P, H, DH + 1], BF16, tag="vall", name="vall")
        V = [vall[:, j] for j in range(NJ // P)]

        # attention pools first: their SBUF must not alias the projection pools
        ctx_pool = octx.enter_context(tc.tile_pool(name="ctxp", bufs=1, side="right"))
        CTX = [ctx_pool.tile([P, NI], BF16, tag=f"ctx{t}", name=f"ctx{t}") for t in range(NDB)]
        esp = octx.enter_context(tc.tile_pool(name="es", bufs=11))
        recp = octx.enter_context(tc.tile_pool(name="rec", bufs=6))
        stp = octx.enter_context(tc.tile_pool(name="stg", bufs=10))

        # ---------------- phase Q (bf16); XQ reused by K; WK/XKB prefetched ----
        xqp = octx.enter_context(tc.tile_pool(name="xq", bufs=1))
        XQ = [xqp.tile([P, NI], BF16, tag=f"xq{c}", name=f"xq{c}") for c in range(NCC)]
        wkp = octx.enter_context(tc.tile_pool(name="wk", bufs=1))
        WK = [wkp.tile([P, D], BF16, tag=f"wk{c}", name=f"wk{c}") for c in range(NCC)]
        with tc.tile_pool(name="wq", bufs=1) as wqp:
            WQ = [wqp.tile([P, D], BF16, tag=f"wq{c}", name=f"wq{c}") for c in range(NCC)]
            # split the first loads so Q chain (db0, ib0) starts ~8us in
            # instead of waiting for all 4MB of interleaved XQ/WQ traffic
            for c in range(NCC):
                nc.sync.dma_start(WQ[c][:, 0 : 2 * P], wqt[c * P : (c + 1) * P, 0 : 2 * P])
            for c in range(NCC):
                nc.sync.dma_start(XQ[c][:, 0:512], xtkv[c * P : (c + 1) * P, 0:512])
            for c in range(NCC):
                nc.sync.dma_start(WQ[c][:, 2 * P : D], wqt[c * P : (c + 1) * P, 2 * P : D])
            for c in range(NCC):
                nc.sync.dma_start(XQ[c][:, 512:NI], xtkv[c * P : (c + 1) * P, 512:NI])
            for c in range(NCC):
                nc.sync.dma_start(WK[c][:], wkt[c * P : (c + 1) * P, :])
            for ib in range(NI // 512):
                for db in range(NDB):
                    ps = psB.tile([P, 512], F32, tag="pj", name="pj")
                    for c in range(NCC):
                        nc.tensor.matmul(
                            ps[:],
                            WQ[c][:, db * P : (db + 1) * P],
                            XQ[c][:, ib * 512 : (ib + 1) * 512],
                            start=(c == 0),
                            stop=(c == NCC - 1),
                        )
                    nc.vector.tensor_copy(QT[db][:, ib * 512 : (ib + 1) * 512], ps[:])

        # Wo pools open before the fused scope so WO/bias prefetch during it
        wop = octx.enter_context(tc.tile_pool(name="wo", bufs=1))
        bip = octx.enter_context(tc.tile_pool(name="bias", bufs=1))
        osp = octx.enter_context(tc.tile_pool(name="os", bufs=6))
        WO = [wop.tile([P, D], BF16, tag=f"wo{f}", name=f"wo{f}") for f in range(NCC)]
        for f in range(NCC):
            nc.sync.dma_start(WO[f][:], wot[f * P : (f + 1) * P, :])
        BIAS = bip.tile([P, D], F32, name="BIAS")
        nc.gpsimd.dma_start(BIAS[:], bo.to_broadcast([P, D]))

        # -------- fused phase K + attention: per db, project KT[db] then emit
        # the two heads (x2 ib blocks) that consume it ------------------------
        with (
            tc.tile_pool(name="xkb", bufs=1) as xkbp,
            tc.tile_pool(name="wvh", bufs=1) as wvhp,
        ):
            XKB = [xkbp.tile([P, NI], BF16, tag=f"xkb{c}", name=f"xkb{c}") for c in range(NCC)]
            XKA = XQ
            for c in range(NCC):
                nc.sync.dma_start(XKB[c][:], xtkv[c * P : (c + 1) * P, NI:NJ])
            for j in range(NJ // P):
                nc.vector.memset(V[j][:, :, DH : DH + 1], 1.0)
            fillers = []

            def v_halfpass_jg(vh, jg):
                """Project V head-half vh for key group jg (4 j-blocks); x is
                sliced straight out of the resident XKA/XKB tiles."""
                for j4 in range(4):
                    j = jg * 4 + j4
                    xh = XKA if j < 8 else XKB
                    jloc = j % 8
                    ps = psB.tile([P, 512], F32, tag="pj", name="pj")
                    for c in range(NCC):
                        nc.tensor.matmul(
                            ps[:],
                            xh[c][:, jloc * P : (jloc + 1) * P],
                            WVH[c][:],
                            start=(c == 0),
                            stop=(c == NCC - 1),
                        )
                    nc.vector.tensor_copy(
                        V[j][:, vh * 8 : (vh + 1) * 8, 0:DH],
                        ps[:].rearrange("p (h d) -> p h d", h=8),
                    )

            WVH = [wvhp.tile([P, 512], BF16, tag=f"wvh{c}", name=f"wvh{c}") for c in range(NCC)]
            # V head-half 0 (heads 0-7): needed from db=0
            for c in range(NCC):
                nc.sync.dma_start(WVH[c][:], wvt[c * P : (c + 1) * P, 0:512])
            for jg in range(NJ // 512):
                v_halfpass_jg(0, jg)

            for db in range(NDB):
                # V head-half 1 (heads 8-15): one key group per db in 1..4
                if db == 1:
                    WVH = [
                        wvhp.tile([P, 512], BF16, tag=f"wvh{c}", name=f"wvh{c}2")
                        for c in range(NCC)
                    ]
                    for c in range(NCC):
                        nc.sync.dma_start(WVH[c][:], wvt[c * P : (c + 1) * P, 512:1024])
                if 1 <= db <= 4:
                    v_halfpass_jg(1, db - 1)
                # K projection for this db
                for jb in range(NJ // 512):
                    half = XKA if jb < 2 else XKB
                    cslc = slice((jb % 2) * 512, (jb % 2) * 512 + 512)
                    ps = psB.tile([P, 512], F32, tag="pj", name="pj")
                    for c in range(NCC):
                        nc.tensor.matmul(
                            ps[:],
                            WK[c][:, db * P : (db + 1) * P],
                            half[c][:, cslc],
                            start=(c == 0),
                            stop=(c == NCC - 1),
                        )
                    nc.vector.tensor_copy(KT[db][:, jb * 512 : (jb + 1) * 512], ps[:])
                # the two heads living in KT[db], for both i-blocks
                t = db
                for ib in range(NI // 512):
                    islc = slice(ib * 512, (ib + 1) * 512)
                    stgs = {}
                    for hh in range(2):
                        h = 2 * db + hh
                        dp = hh * DH
                        es_list = []
                        for pr in range(NJ // 256):
                            sp = psA.tile([P, 1024], F32, tag="sp", name="sp")
                            for half2 in range(2):
                                j = pr * 2 + half2
                                nc.tensor.matmul(
                                    sp[:, half2 * 512 : (half2 + 1) * 512],
                                    KT[t][dp : dp + DH, j * P : (j + 1) * P],
                                    QT[t][dp : dp + DH, islc],
                                    start=True,
                                    stop=True,
                                )
                            es = esp.tile([P, 1024], BF16, tag="es", name="es")
                            nc.scalar.activation(es[:], sp[:], EXP, scale=SCALE)
                            es_list.append(es)
                            if fillers:
                                fillers.pop(0)()
                        # AV with es STATIONARY and V|1 MOVING: N=65 cycles per
                        # matmul instead of 512 -- output [i, d|sum] per i-128
                        # block, so the softmax divide is a native per-partition
                        # tensor_scalar, and a bf16 DMA-transpose restores the
                        # [f, i] layout Wo consumes.
                        for q in range(4):
                            ctp = psC.tile([P, DH + 1], F32, tag="ct", name="ct")
                            for j in range(NJ // P):
                                nc.tensor.matmul(
                                    ctp[:],
                                    es_list[j // 2][
                                        :,
                                        (j % 2) * 512 + q * P : (j % 2) * 512 + (q + 1) * P,
                                    ],
                                    V[j][:, h, :],
                                    start=(j == 0),
                                    stop=(j == NJ // P - 1),
                                )
                            rec = recp.tile([P, 1], F32, tag="rec", name="rec")
                            nc.vector.reciprocal(rec[:], ctp[:, DH : DH + 1])
                            if hh == 0:
                                stgs[q] = stp.tile([P, 2 * DH], BF16, tag="st", name="st")
                            stg = stgs[q]
                            nc.vector.tensor_scalar_mul(
                                stg[:, dp : dp + DH], ctp[:, 0:DH], rec[:]
                            )
                            if hh == 1:
                                # both heads of the pair staged: one 128-wide
                                # bf16 DMA-transpose fills CTX tile t's columns
                                nc.sync.dma_start_transpose(
                                    CTX[t][:, ib * 512 + q * P : ib * 512 + (q + 1) * P],
                                    stg[:],
                                )
                if db in (4, 6):
                    # partial Wo over the ctx tiles finished so far, woven as
                    # fillers between the next db's score tiles; partials
                    # (+ bias) park in QT tiles whose scores are done.
                    lo, hi = (0, 4) if db == 4 else (4, 7)
                    nf = 5 if db == 4 else 7

                    def wo_part(ib8, eb, nf):
                        ps = psB.tile([P, 512], F32, tag="pj", name="pj")
                        for f in range(nf):
                            nc.tensor.matmul(
                                ps[:],
                                CTX[f][:, ib8 * P : (ib8 + 1) * P],
                                WO[f][:, eb * 512 : (eb + 1) * 512],
                                start=(f == 0),
                                stop=(f == nf - 1),
                            )
                        nc.vector.tensor_add(
                            QT[ib8][:, eb * 512 : (eb + 1) * 512],
                            ps[:],
                            BIAS[:, eb * 512 : (eb + 1) * 512],
                        )

                    fillers += [
                        lambda ib8=ib8, eb=eb, nf=nf: wo_part(ib8, eb, nf)
                        for ib8 in range(lo, hi)
                        for eb in range(2)
                    ]

        if DEBUG:
            nc.sync.dma_start(dbg_ctx0, CTX[0][:])

        # ---------------- phase Wo, final pass ----------------------------
        # blocks ib8<=6 were partially accumulated into dead QT tiles during
        # the ACT-throttled late dbs; finish their remaining chunks here.
        for ib8 in range(NI // P):
            f0 = 5 if ib8 <= 3 else (7 if ib8 <= 6 else 0)
            for eb in range(2):
                ps = psB.tile([P, 512], F32, tag="pj", name="pj")
                for f in range(f0, NCC):
                    nc.tensor.matmul(
                        ps[:],
                        CTX[f][:, ib8 * P : (ib8 + 1) * P],
                        WO[f][:, eb * 512 : (eb + 1) * 512],
                        start=(f == f0),
                        stop=(f == NCC - 1),
                    )
                ostage = osp.tile([P, 512], BF16, tag="os", name="os")
                prev = (
                    QT[ib8][:, eb * 512 : (eb + 1) * 512]
                    if ib8 <= 6
                    else BIAS[:, eb * 512 : (eb + 1) * 512]
                )
                nc.vector.tensor_add(ostage[:], ps[:], prev)
                nc.sync.dma_start(
                    out[ib8 * P : (ib8 + 1) * P, eb * 512 : (eb + 1) * 512],
                    ostage[:],
                )

    nc.compile()
    return nc


_NC = None


def _get_nc():
    global _NC
    if _NC is None:
        _NC = _build()
    return _NC


def _make_in_maps(x, Wq, Wk, Wv, Wo, bo):
    import ml_dtypes

    bf16 = ml_dtypes.bfloat16
    wqt = np.ascontiguousarray(Wq.T).astype(bf16)
    wkt = np.ascontiguousarray(Wk.T).astype(bf16)
    wvt = np.ascontiguousarray(Wv.T).astype(bf16)
    wot = np.ascontiguousarray(Wo.T).astype(bf16)
    bo2 = np.ascontiguousarray(bo.reshape(1, D)).astype(np.float32)
    in_maps = []
    for c in range(NCORES):
        b, s = c // 2, c % 2
        mine = x[b, s * NI : (s + 1) * NI, :].T
        other = x[b, (1 - s) * NI : (2 - s) * NI, :].T
        xtkv = np.ascontiguousarray(np.concatenate([mine, other], axis=1)).astype(bf16)
        in_maps.append(
            {"xtkv": xtkv, "wqt": wqt, "wkt": wkt, "wvt": wvt, "wot": wot, "bo": bo2}
        )
    return in_maps


def _run(x, Wq, Wk, Wv, Wo, bo, **spmd_kwargs):
    nc = _get_nc()
    in_maps = _make_in_maps(x, Wq, Wk, Wv, Wo, bo)
    res = run_bass_kernel_spmd(nc, in_maps, list(range(NCORES)), **spmd_kwargs)
    outs = [np.asarray(res.results[c]["out"]) for c in range(NCORES)]
    full = np.concatenate(outs, axis=0).reshape(4, 2048, D).astype(np.float32)
    return full, res


def kernel(x, Wq, Wk, Wv, Wo, bo):
    full, _ = _run(
        np.asarray(x), np.asarray(Wq), np.asarray(Wk), np.asarray(Wv),
        np.asarray(Wo), np.asarray(bo),
    )
    return full



# revision 72
# speedup vs baseline: 1.0705x; 1.0092x over previous
"""Multi-head attention (b=4, n=2048, d=1024, h=16, dh=64) on 8 TRN2 NeuronCores.

Sharding: batch x sequence-half per core (core c handles batch b=c//2, query
rows s=(c%2)*1024 .. +1024). Each core recomputes K/V for its whole batch
locally (no collectives), computes flash-style attention for its 1024 query
rows over all 16 heads, applies the output projection, and writes a disjoint
1024-row slice of the flattened output. Matmuls run in fp32r (TF32-like fast
fp32 mode); the attention AV product runs in bf16.

Host-side layout choices (free transposes/permutes in numpy):
  xtkv [d, 2048] = concat(x[b, my_half].T, x[b, other_half].T) -- the core's
      own query rows are ALWAYS columns 0:1024, so the same SPMD graph works
      on every core, and key order permutation is softmax-invariant.
  wqt/wkt/wvt/wot = W.T (contraction dim first), bo as [1, d].
"""

import sys

sys.path.insert(0, "/opt/trn_rl_repo")

from contextlib import ExitStack

import numpy as np

import concourse.bass as bass
import concourse.tile as tile
from concourse import bacc, mybir
from concourse.bass_utils import run_bass_kernel_spmd

F32 = mybir.dt.float32
F32R = mybir.dt.float32r
BF16 = mybir.dt.bfloat16
EXP = mybir.ActivationFunctionType.Exp

P = 128
D = 1024  # model dim
NI = 1024  # query rows per core
NJ = 2048  # key rows per core (full batch)
H = 16  # heads
DH = 64  # head dim
SCALE = DH**-0.5  # 0.125
NCORES = 8

NCC = D // P  # 8 contraction chunks
NDB = D // P  # 8 feature blocks


def _build():
    nc = bacc.Bacc("TRN2", target_bir_lowering=False, debug=False, num_devices=NCORES)

    xtkv = nc.dram_tensor("xtkv", [D, NJ], BF16, kind="ExternalInput").ap()
    wqt = nc.dram_tensor("wqt", [D, D], BF16, kind="ExternalInput").ap()
    wkt = nc.dram_tensor("wkt", [D, D], BF16, kind="ExternalInput").ap()
    wvt = nc.dram_tensor("wvt", [D, D], BF16, kind="ExternalInput").ap()
    wot = nc.dram_tensor("wot", [D, D], BF16, kind="ExternalInput").ap()
    bo = nc.dram_tensor("bo", [1, D], F32, kind="ExternalInput").ap()
    out = nc.dram_tensor("out", [NI, D], BF16, kind="ExternalOutput").ap()
    DEBUG = bool(__import__("os").environ.get("KERNEL_DEBUG"))
    if DEBUG:
        dbg_kt0 = nc.dram_tensor("dbg_kt0", [P, NJ], BF16, kind="ExternalOutput").ap()
        dbg_qt0 = nc.dram_tensor("dbg_qt0", [P, NI], BF16, kind="ExternalOutput").ap()
        dbg_v0 = nc.dram_tensor("dbg_v0", [P, H, DH + 1], BF16, kind="ExternalOutput").ap()
        dbg_ctx0 = nc.dram_tensor("dbg_ctx0", [P, NI], BF16, kind="ExternalOutput").ap()

    with tile.TileContext(nc) as tc, ExitStack() as octx:
        # kernel-wide PSUM pools: 4 + 2 + 2 = 8 banks
        psA = octx.enter_context(tc.tile_pool(name="psA", bufs=2, space="PSUM"))
        psB = octx.enter_context(tc.tile_pool(name="psB", bufs=2, space="PSUM"))
        psC = octx.enter_context(tc.tile_pool(name="psC", bufs=2, space="PSUM"))
        dramp = octx.enter_context(tc.tile_pool(name="dramp", bufs=4, space="DRAM"))

        kt_pool = octx.enter_context(tc.tile_pool(name="ktp", bufs=1))
        qt_pool = octx.enter_context(tc.tile_pool(name="qtp", bufs=1))
        v_pool = octx.enter_context(tc.tile_pool(name="vp", bufs=1))
        KT = [kt_pool.tile([P, NJ], BF16, tag=f"kt{i}", name=f"kt{i}") for i in range(NDB)]
        QT = [qt_pool.tile([P, NI], BF16, tag=f"qt{i}", name=f"qt{i}") for i in range(NDB)]
        vall = v_pool.tile([P, NJ // P, H, DH + 1], BF16, tag="vall", name="vall")
        V = [vall[:, j] for j in range(NJ // P)]

        # attention pools first: their SBUF must not alias the projection pools
        ctx_pool = octx.enter_context(tc.tile_pool(name="ctxp", bufs=1, side="right"))
        CTX = [ctx_pool.tile([P, NI], BF16, tag=f"ctx{t}", name=f"ctx{t}") for t in range(NDB)]
        esp = octx.enter_context(tc.tile_pool(name="es", bufs=11))
        recp = octx.enter_context(tc.tile_pool(name="rec", bufs=6))
        stp = octx.enter_context(tc.tile_pool(name="stg", bufs=10))

        # ---------------- phase Q (bf16); XQ reused by K; WK/XKB prefetched ----
        xqp = octx.enter_context(tc.tile_pool(name="xq", bufs=1))
        XQ = [xqp.tile([P, NI], BF16, tag=f"xq{c}", name=f"xq{c}") for c in range(NCC)]
        wkp = octx.enter_context(tc.tile_pool(name="wk", bufs=1))
        WK = [wkp.tile([P, D], BF16, tag=f"wk{c}", name=f"wk{c}") for c in range(NCC)]
        with tc.tile_pool(name="wq", bufs=1) as wqp:
            WQ = [wqp.tile([P, D], BF16, tag=f"wq{c}", name=f"wq{c}") for c in range(NCC)]
            # split the first loads so Q chain (db0, ib0) starts ~8us in
            # instead of waiting for all 4MB of interleaved XQ/WQ traffic
            for c in range(NCC):
                nc.sync.dma_start(WQ[c][:, 0 : 2 * P], wqt[c * P : (c + 1) * P, 0 : 2 * P])
            for c in range(NCC):
                nc.sync.dma_start(XQ[c][:, 0:512], xtkv[c * P : (c + 1) * P, 0:512])
            for c in range(NCC):
                nc.sync.dma_start(WQ[c][:, 2 * P : D], wqt[c * P : (c + 1) * P, 2 * P : D])
            for c in range(NCC):
                nc.sync.dma_start(XQ[c][:, 512:NI], xtkv[c * P : (c + 1) * P, 512:NI])
            for c in range(NCC):
                nc.sync.dma_start(WK[c][:], wkt[c * P : (c + 1) * P, :])
            for ib in range(NI // 512):
                for db in range(NDB):
                    ps = psB.tile([P, 512], F32, tag="pj", name="pj")
                    for c in range(NCC):
                        nc.tensor.matmul(
                            ps[:],
                            WQ[c][:, db * P : (db + 1) * P],
                            XQ[c][:, ib * 512 : (ib + 1) * 512],
                            start=(c == 0),
                            stop=(c == NCC - 1),
                        )
                    nc.vector.tensor_copy(QT[db][:, ib * 512 : (ib + 1) * 512], ps[:])

        # Wo pools open before the fused scope so WO/bias prefetch during it
        wop = octx.enter_context(tc.tile_pool(name="wo", bufs=1))
        bip = octx.enter_context(tc.tile_pool(name="bias", bufs=1))
        osp = octx.enter_context(tc.tile_pool(name="os", bufs=6))
        WO = [wop.tile([P, D], BF16, tag=f"wo{f}", name=f"wo{f}") for f in range(NCC)]
        for f in range(NCC):
            nc.sync.dma_start(WO[f][:], wot[f * P : (f + 1) * P, :])
        BIAS = bip.tile([P, D], F32, name="BIAS")
        nc.gpsimd.dma_start(BIAS[:], bo.to_broadcast([P, D]))

        # -------- fused phase K + attention: per db, project KT[db] then emit
        # the two heads (x2 ib blocks) that consume it ------------------------
        with (
            tc.tile_pool(name="xkb", bufs=1) as xkbp,
            tc.tile_pool(name="wvh", bufs=1) as wvhp,
        ):
            XKB = [xkbp.tile([P, NI], BF16, tag=f"xkb{c}", name=f"xkb{c}") for c in range(NCC)]
            XKA = XQ
            for c in range(NCC):
                nc.sync.dma_start(XKB[c][:], xtkv[c * P : (c + 1) * P, NI:NJ])
            for j in range(NJ // P):
                nc.vector.memset(V[j][:, :, DH : DH + 1], 1.0)
            fillers = []

            def v_halfpass_jg(vh, jg):
                """Project V head-half vh for key group jg (4 j-blocks); x is
                sliced straight out of the resident XKA/XKB tiles."""
                for j4 in range(4):
                    j = jg * 4 + j4
                    xh = XKA if j < 8 else XKB
                    jloc = j % 8
                    ps = psB.tile([P, 512], F32, tag="pj", name="pj")
                    for c in range(NCC):
                        nc.tensor.matmul(
                            ps[:],
                            xh[c][:, jloc * P : (jloc + 1) * P],
                            WVH[c][:],
                            start=(c == 0),
                            stop=(c == NCC - 1),
                        )
                    nc.vector.tensor_copy(
                        V[j][:, vh * 8 : (vh + 1) * 8, 0:DH],
                        ps[:].rearrange("p (h d) -> p h d", h=8),
                    )

            WVH = [wvhp.tile([P, 512], BF16, tag=f"wvh{c}", name=f"wvh{c}") for c in range(NCC)]
            # V head-half 0 (heads 0-7): needed from db=0
            for c in range(NCC):
                nc.sync.dma_start(WVH[c][:], wvt[c * P : (c + 1) * P, 0:512])
            for jg in range(NJ // 512):
                v_halfpass_jg(0, jg)

            for db in range(NDB):
                # V head-half 1 (heads 8-15): one key group per db in 1..4
                if db == 1:
                    WVH = [
                        wvhp.tile([P, 512], BF16, tag=f"wvh{c}", name=f"wvh{c}2")
                        for c in range(NCC)
                    ]
                    for c in range(NCC):
                        nc.sync.dma_start(WVH[c][:], wvt[c * P : (c + 1) * P, 512:1024])
                if 1 <= db <= 4:
                    v_halfpass_jg(1, db - 1)
                # K projection for this db
                for jb in range(NJ // 512):
                    half = XKA if jb < 2 else XKB
                    cslc = slice((jb % 2) * 512, (jb % 2) * 512 + 512)
                    ps = psB.tile([P, 512], F32, tag="pj", name="pj")
                    for c in range(NCC):
                        nc.tensor.matmul(
                            ps[:],
                            WK[c][:, db * P : (db + 1) * P],
                            half[c][:, cslc],
                            start=(c == 0),
                            stop=(c == NCC - 1),
                        )
                    nc.vector.tensor_copy(KT[db][:, jb * 512 : (jb + 1) * 512], ps[:])
                # the two heads living in KT[db], for both i-blocks
                t = db
                for ib in range(NI // 512):
                    islc = slice(ib * 512, (ib + 1) * 512)
                    stgs = {}
                    for hh in range(2):
                        h = 2 * db + hh
                        dp = hh * DH
                        es_list = []
                        for pr in range(NJ // 256):
                            sp = psA.tile([P, 1024], F32, tag="sp", name="sp")
                            for half2 in range(2):
                                j = pr * 2 + half2
                                nc.tensor.matmul(
                                    sp[:, half2 * 512 : (half2 + 1) * 512],
                                    KT[t][dp : dp + DH, j * P : (j + 1) * P],
                                    QT[t][dp : dp + DH, islc],
                                    start=True,
                                    stop=True,
                                )
                            es = esp.tile([P, 1024], BF16, tag="es", name="es")
                            nc.scalar.activation(es[:], sp[:], EXP, scale=SCALE)
                            es_list.append(es)
                            if fillers:
                                fillers.pop(0)()
                        # AV with es STATIONARY and V|1 MOVING: N=65 cycles per
                        # matmul instead of 512 -- output [i, d|sum] per i-128
                        # block, so the softmax divide is a native per-partition
                        # tensor_scalar, and a bf16 DMA-transpose restores the
                        # [f, i] layout Wo consumes.
                        for q in range(4):
                            ctp = psC.tile([P, DH + 1], F32, tag="ct", name="ct")
                            for j in range(NJ // P):
                                nc.tensor.matmul(
                                    ctp[:],
                                    es_list[j // 2][
                                        :,
                                        (j % 2) * 512 + q * P : (j % 2) * 512 + (q + 1) * P,
                                    ],
                                    V[j][:, h, :],
                                    start=(j == 0),
                                    stop=(j == NJ // P - 1),
                                )
                            rec = recp.tile([P, 1], F32, tag="rec", name="rec")
                            nc.vector.reciprocal(rec[:], ctp[:, DH : DH + 1])
                            if hh == 0:
                                stgs[q] = stp.tile([P, 2 * DH], BF16, tag="st", name="st")
                            stg = stgs[q]
                            nc.vector.tensor_scalar_mul(
                                stg[:, dp : dp + DH], ctp[:, 0:DH], rec[:]
                            )
                            if hh == 1:
                                # both heads of the pair staged: one 128-wide
                                # bf16 DMA-transpose fills CTX tile t's columns
                                nc.sync.dma_start_transpose(
                                    CTX[t][:, ib * 512 + q * P : ib * 512 + (q + 1) * P],
                                    stg[:],
                                )
                if db == 5:
                    # one more chunk for blocks 0..3 (RMW into the parked
                    # partials), woven into db6's score stream
                    def wo_rmw(ib8, eb, f):
                        ps = psB.tile([P, 512], F32, tag="pj", name="pj")
                        nc.tensor.matmul(
                            ps[:],
                            CTX[f][:, ib8 * P : (ib8 + 1) * P],
                            WO[f][:, eb * 512 : (eb + 1) * 512],
                            start=True,
                            stop=True,
                        )
                        dst = QT[ib8][:, eb * 512 : (eb + 1) * 512]
                        nc.vector.tensor_add(dst, ps[:], dst)

                    fillers += [
                        lambda ib8=ib8, eb=eb: wo_rmw(ib8, eb, 5)
                        for ib8 in range(4)
                        for eb in range(2)
                    ]
                if db == 6:
                    fillers += [
                        lambda ib8=ib8, eb=eb: wo_rmw(ib8, eb, 6)
                        for ib8 in range(4)
                        for eb in range(2)
                    ]
                if db in (4, 6):
                    # partial Wo over the ctx tiles finished so far, woven as
                    # fillers between the next db's score tiles; partials
                    # (+ bias) park in QT tiles whose scores are done.
                    lo, hi = (0, 4) if db == 4 else (4, 7)
                    nf = 5 if db == 4 else 7

                    def wo_part(ib8, eb, nf):
                        ps = psB.tile([P, 512], F32, tag="pj", name="pj")
                        for f in range(nf):
                            nc.tensor.matmul(
                                ps[:],
                                CTX[f][:, ib8 * P : (ib8 + 1) * P],
                                WO[f][:, eb * 512 : (eb + 1) * 512],
                                start=(f == 0),
                                stop=(f == nf - 1),
                            )
                        nc.vector.tensor_add(
                            QT[ib8][:, eb * 512 : (eb + 1) * 512],
                            ps[:],
                            BIAS[:, eb * 512 : (eb + 1) * 512],
                        )

                    fillers += [
                        lambda ib8=ib8, eb=eb, nf=nf: wo_part(ib8, eb, nf)
                        for ib8 in range(lo, hi)
                        for eb in range(2)
                    ]

        if DEBUG:
            nc.sync.dma_start(dbg_ctx0, CTX[0][:])

        # ---------------- phase Wo, final pass ----------------------------
        # blocks ib8<=6 were partially accumulated into dead QT tiles during
        # the ACT-throttled late dbs; finish their remaining chunks here.
        for ib8 in range(NI // P):
            f0 = 7 if ib8 <= 6 else 0
            for eb in range(2):
                ps = psB.tile([P, 512], F32, tag="pj", name="pj")
                for f in range(f0, NCC):
                    nc.tensor.matmul(
                        ps[:],
                        CTX[f][:, ib8 * P : (ib8 + 1) * P],
                        WO[f][:, eb * 512 : (eb + 1) * 512],
                        start=(f == f0),
                        stop=(f == NCC - 1),
                    )
                ostage = osp.tile([P, 512], BF16, tag="os", name="os")
                prev = (
                    QT[ib8][:, eb * 512 : (eb + 1) * 512]
                    if ib8 <= 6
                    else BIAS[:, eb * 512 : (eb + 1) * 512]
                )
                nc.vector.tensor_add(ostage[:], ps[:], prev)
                nc.sync.dma_start(
                    out[ib8 * P : (ib8 + 1) * P, eb * 512 : (eb + 1) * 512],
                    ostage[:],
                )

    nc.compile()
    return nc


_NC = None


def _get_nc():
    global _NC
    if _NC is None:
        _NC = _build()
    return _NC


def _make_in_maps(x, Wq, Wk, Wv, Wo, bo):
    import ml_dtypes

    bf16 = ml_dtypes.bfloat16
    wqt = np.ascontiguousarray(Wq.T).astype(bf16)
    wkt = np.ascontiguousarray(Wk.T).astype(bf16)
    wvt = np.ascontiguousarray(Wv.T).astype(bf16)
    wot = np.ascontiguousarray(Wo.T).astype(bf16)
    bo2 = np.ascontiguousarray(bo.reshape(1, D)).astype(np.float32)
    in_maps = []
    for c in range(NCORES):
        b, s = c // 2, c % 2
        mine = x[b, s * NI : (s + 1) * NI, :].T
        other = x[b, (1 - s) * NI : (2 - s) * NI, :].T
        xtkv = np.ascontiguousarray(np.concatenate([mine, other], axis=1)).astype(bf16)
        in_maps.append(
            {"xtkv": xtkv, "wqt": wqt, "wkt": wkt, "wvt": wvt, "wot": wot, "bo": bo2}
        )
    return in_maps


def _run(x, Wq, Wk, Wv, Wo, bo, **spmd_kwargs):
    nc = _get_nc()
    in_maps = _make_in_maps(x, Wq, Wk, Wv, Wo, bo)
    res = run_bass_kernel_spmd(nc, in_maps, list(range(NCORES)), **spmd_kwargs)
    outs = [np.asarray(res.results[c]["out"]) for c in range(NCORES)]
    full = np.concatenate(outs, axis=0).reshape(4, 2048, D).astype(np.float32)
    return full, res


def kernel(x, Wq, Wk, Wv, Wo, bo):
    full, _ = _run(
        np.asarray(x), np.asarray(Wq), np.asarray(Wk), np.asarray(Wv),
        np.asarray(Wo), np.asarray(bo),
    )
    return full



# revision 75
# speedup vs baseline: 1.0725x; 1.0019x over previous
"""Multi-head attention (b=4, n=2048, d=1024, h=16, dh=64) on 8 TRN2 NeuronCores.

Sharding: batch x sequence-half per core (core c handles batch b=c//2, query
rows s=(c%2)*1024 .. +1024). Each core recomputes K/V for its whole batch
locally (no collectives), computes flash-style attention for its 1024 query
rows over all 16 heads, applies the output projection, and writes a disjoint
1024-row slice of the flattened output. Matmuls run in fp32r (TF32-like fast
fp32 mode); the attention AV product runs in bf16.

Host-side layout choices (free transposes/permutes in numpy):
  xtkv [d, 2048] = concat(x[b, my_half].T, x[b, other_half].T) -- the core's
      own query rows are ALWAYS columns 0:1024, so the same SPMD graph works
      on every core, and key order permutation is softmax-invariant.
  wqt/wkt/wvt/wot = W.T (contraction dim first), bo as [1, d].
"""

import sys

sys.path.insert(0, "/opt/trn_rl_repo")

from contextlib import ExitStack

import numpy as np

import concourse.bass as bass
import concourse.tile as tile
from concourse import bacc, mybir
from concourse.bass_utils import run_bass_kernel_spmd

F32 = mybir.dt.float32
F32R = mybir.dt.float32r
BF16 = mybir.dt.bfloat16
EXP = mybir.ActivationFunctionType.Exp

P = 128
D = 1024  # model dim
NI = 1024  # query rows per core
NJ = 2048  # key rows per core (full batch)
H = 16  # heads
DH = 64  # head dim
SCALE = DH**-0.5  # 0.125
NCORES = 8

NCC = D // P  # 8 contraction chunks
NDB = D // P  # 8 feature blocks


def _build():
    nc = bacc.Bacc("TRN2", target_bir_lowering=False, debug=False, num_devices=NCORES)

    xtkv = nc.dram_tensor("xtkv", [D, NJ], BF16, kind="ExternalInput").ap()
    wqt = nc.dram_tensor("wqt", [D, D], BF16, kind="ExternalInput").ap()
    wkt = nc.dram_tensor("wkt", [D, D], BF16, kind="ExternalInput").ap()
    wvt = nc.dram_tensor("wvt", [D, D], BF16, kind="ExternalInput").ap()
    wot = nc.dram_tensor("wot", [D, D], BF16, kind="ExternalInput").ap()
    bo = nc.dram_tensor("bo", [1, D], F32, kind="ExternalInput").ap()
    out = nc.dram_tensor("out", [NI, D], BF16, kind="ExternalOutput").ap()
    DEBUG = bool(__import__("os").environ.get("KERNEL_DEBUG"))
    if DEBUG:
        dbg_kt0 = nc.dram_tensor("dbg_kt0", [P, NJ], BF16, kind="ExternalOutput").ap()
        dbg_qt0 = nc.dram_tensor("dbg_qt0", [P, NI], BF16, kind="ExternalOutput").ap()
        dbg_v0 = nc.dram_tensor("dbg_v0", [P, H, DH + 1], BF16, kind="ExternalOutput").ap()
        dbg_ctx0 = nc.dram_tensor("dbg_ctx0", [P, NI], BF16, kind="ExternalOutput").ap()

    with tile.TileContext(nc) as tc, ExitStack() as octx:
        # kernel-wide PSUM pools: 4 + 2 + 2 = 8 banks
        psA = octx.enter_context(tc.tile_pool(name="psA", bufs=2, space="PSUM"))
        psB = octx.enter_context(tc.tile_pool(name="psB", bufs=2, space="PSUM"))
        psC = octx.enter_context(tc.tile_pool(name="psC", bufs=2, space="PSUM"))
        dramp = octx.enter_context(tc.tile_pool(name="dramp", bufs=4, space="DRAM"))

        kt_pool = octx.enter_context(tc.tile_pool(name="ktp", bufs=1))
        qt_pool = octx.enter_context(tc.tile_pool(name="qtp", bufs=1))
        v_pool = octx.enter_context(tc.tile_pool(name="vp", bufs=1))
        KT = [kt_pool.tile([P, NJ], BF16, tag=f"kt{i}", name=f"kt{i}") for i in range(NDB)]
        QT = [qt_pool.tile([P, NI], BF16, tag=f"qt{i}", name=f"qt{i}") for i in range(NDB)]
        vall = v_pool.tile([P, NJ // P, H, DH + 1], BF16, tag="vall", name="vall")
        V = [vall[:, j] for j in range(NJ // P)]

        # attention pools first: their SBUF must not alias the projection pools
        ctx_pool = octx.enter_context(tc.tile_pool(name="ctxp", bufs=1, side="right"))
        CTX = [ctx_pool.tile([P, NI], BF16, tag=f"ctx{t}", name=f"ctx{t}") for t in range(NDB)]
        esp = octx.enter_context(tc.tile_pool(name="es", bufs=11))
        recp = octx.enter_context(tc.tile_pool(name="rec", bufs=6))
        stp = octx.enter_context(tc.tile_pool(name="stg", bufs=10))

        # ---------------- phase Q (bf16); XQ reused by K; WK/XKB prefetched ----
        xqp = octx.enter_context(tc.tile_pool(name="xq", bufs=1))
        XQ = [xqp.tile([P, NI], BF16, tag=f"xq{c}", name=f"xq{c}") for c in range(NCC)]
        wkp = octx.enter_context(tc.tile_pool(name="wk", bufs=1))
        WK = [wkp.tile([P, D], BF16, tag=f"wk{c}", name=f"wk{c}") for c in range(NCC)]
        with tc.tile_pool(name="wq", bufs=1) as wqp:
            WQ = [wqp.tile([P, D], BF16, tag=f"wq{c}", name=f"wq{c}") for c in range(NCC)]
            # split the first loads so Q chain (db0, ib0) starts ~8us in
            # instead of waiting for all 4MB of interleaved XQ/WQ traffic
            for c in range(NCC):
                nc.sync.dma_start(WQ[c][:, 0 : 2 * P], wqt[c * P : (c + 1) * P, 0 : 2 * P])
            for c in range(NCC):
                nc.sync.dma_start(XQ[c][:, 0:512], xtkv[c * P : (c + 1) * P, 0:512])
            for c in range(NCC):
                nc.sync.dma_start(WQ[c][:, 2 * P : D], wqt[c * P : (c + 1) * P, 2 * P : D])
            for c in range(NCC):
                nc.sync.dma_start(XQ[c][:, 512:NI], xtkv[c * P : (c + 1) * P, 512:NI])
            for c in range(NCC):
                nc.sync.dma_start(WK[c][:], wkt[c * P : (c + 1) * P, :])
            for ib in range(NI // 512):
                for db in range(NDB):
                    ps = psB.tile([P, 512], F32, tag="pj", name="pj")
                    for c in range(NCC):
                        nc.tensor.matmul(
                            ps[:],
                            WQ[c][:, db * P : (db + 1) * P],
                            XQ[c][:, ib * 512 : (ib + 1) * 512],
                            start=(c == 0),
                            stop=(c == NCC - 1),
                        )
                    nc.vector.tensor_copy(QT[db][:, ib * 512 : (ib + 1) * 512], ps[:])

        # Wo pools open before the fused scope so WO/bias prefetch during it
        wop = octx.enter_context(tc.tile_pool(name="wo", bufs=1))
        bip = octx.enter_context(tc.tile_pool(name="bias", bufs=1))
        osp = octx.enter_context(tc.tile_pool(name="os", bufs=6))
        WO = [wop.tile([P, D], BF16, tag=f"wo{f}", name=f"wo{f}") for f in range(NCC)]
        for f in range(NCC):
            nc.sync.dma_start(WO[f][:], wot[f * P : (f + 1) * P, :])
        BIAS = bip.tile([P, D], F32, name="BIAS")
        nc.gpsimd.dma_start(BIAS[:], bo.to_broadcast([P, D]))

        # -------- fused phase K + attention: per db, project KT[db] then emit
        # the two heads (x2 ib blocks) that consume it ------------------------
        with (
            tc.tile_pool(name="xkb", bufs=1) as xkbp,
            tc.tile_pool(name="wvh", bufs=1) as wvhp,
        ):
            XKB = [xkbp.tile([P, NI], BF16, tag=f"xkb{c}", name=f"xkb{c}") for c in range(NCC)]
            XKA = XQ
            for c in range(NCC):
                nc.sync.dma_start(XKB[c][:], xtkv[c * P : (c + 1) * P, NI:NJ])
            for j in range(NJ // P):
                nc.vector.memset(V[j][:, :, DH : DH + 1], 1.0)
            fillers = []

            def v_halfpass_jg(vh, jg):
                """Project V head-half vh for key group jg (4 j-blocks); x is
                sliced straight out of the resident XKA/XKB tiles."""
                for j4 in range(4):
                    j = jg * 4 + j4
                    xh = XKA if j < 8 else XKB
                    jloc = j % 8
                    ps = psB.tile([P, 512], F32, tag="pj", name="pj")
                    for c in range(NCC):
                        nc.tensor.matmul(
                            ps[:],
                            xh[c][:, jloc * P : (jloc + 1) * P],
                            WVH[c][:],
                            start=(c == 0),
                            stop=(c == NCC - 1),
                        )
                    nc.vector.tensor_copy(
                        V[j][:, vh * 8 : (vh + 1) * 8, 0:DH],
                        ps[:].rearrange("p (h d) -> p h d", h=8),
                    )

            WVH = [wvhp.tile([P, 512], BF16, tag=f"wvh{c}", name=f"wvh{c}") for c in range(NCC)]
            # V head-half 0 (heads 0-7): needed from db=0
            for c in range(NCC):
                nc.sync.dma_start(WVH[c][:], wvt[c * P : (c + 1) * P, 0:512])
            for jg in range(NJ // 512):
                v_halfpass_jg(0, jg)

            for db in range(NDB):
                # V head-half 1 (heads 8-15): one key group per db in 1..4
                if db == 1:
                    WVH = [
                        wvhp.tile([P, 512], BF16, tag=f"wvh{c}", name=f"wvh{c}2")
                        for c in range(NCC)
                    ]
                    for c in range(NCC):
                        nc.sync.dma_start(WVH[c][:], wvt[c * P : (c + 1) * P, 512:1024])
                if 1 <= db <= 4:
                    v_halfpass_jg(1, db - 1)
                # K projection for this db
                for jb in range(NJ // 512):
                    half = XKA if jb < 2 else XKB
                    cslc = slice((jb % 2) * 512, (jb % 2) * 512 + 512)
                    ps = psB.tile([P, 512], F32, tag="pj", name="pj")
                    for c in range(NCC):
                        nc.tensor.matmul(
                            ps[:],
                            WK[c][:, db * P : (db + 1) * P],
                            half[c][:, cslc],
                            start=(c == 0),
                            stop=(c == NCC - 1),
                        )
                    nc.vector.tensor_copy(KT[db][:, jb * 512 : (jb + 1) * 512], ps[:])
                # the two heads living in KT[db], for both i-blocks
                t = db
                for ib in range(NI // 512):
                    islc = slice(ib * 512, (ib + 1) * 512)
                    stgs = {}
                    for hh in range(2):
                        h = 2 * db + hh
                        dp = hh * DH
                        es_list = []
                        for pr in range(NJ // 256):
                            sp = psA.tile([P, 1024], F32, tag="sp", name="sp")
                            for half2 in range(2):
                                j = pr * 2 + half2
                                nc.tensor.matmul(
                                    sp[:, half2 * 512 : (half2 + 1) * 512],
                                    KT[t][dp : dp + DH, j * P : (j + 1) * P],
                                    QT[t][dp : dp + DH, islc],
                                    start=True,
                                    stop=True,
                                )
                            es = esp.tile([P, 1024], BF16, tag="es", name="es")
                            nc.scalar.activation(es[:], sp[:], EXP, scale=SCALE)
                            es_list.append(es)
                            if fillers:
                                fillers.pop(0)()
                        # AV with es STATIONARY and V|1 MOVING: N=65 cycles per
                        # matmul instead of 512 -- output [i, d|sum] per i-128
                        # block, so the softmax divide is a native per-partition
                        # tensor_scalar, and a bf16 DMA-transpose restores the
                        # [f, i] layout Wo consumes.
                        for q in range(4):
                            ctp = psC.tile([P, DH + 1], F32, tag="ct", name="ct")
                            for j in range(NJ // P):
                                nc.tensor.matmul(
                                    ctp[:],
                                    es_list[j // 2][
                                        :,
                                        (j % 2) * 512 + q * P : (j % 2) * 512 + (q + 1) * P,
                                    ],
                                    V[j][:, h, :],
                                    start=(j == 0),
                                    stop=(j == NJ // P - 1),
                                )
                            rec = recp.tile([P, 1], F32, tag="rec", name="rec")
                            nc.vector.reciprocal(rec[:], ctp[:, DH : DH + 1])
                            if hh == 0:
                                stgs[q] = stp.tile([P, 2 * DH], BF16, tag="st", name="st")
                            stg = stgs[q]
                            nc.vector.tensor_scalar_mul(
                                stg[:, dp : dp + DH], ctp[:, 0:DH], rec[:]
                            )
                            if hh == 1:
                                # both heads of the pair staged: one 128-wide
                                # bf16 DMA-transpose fills CTX tile t's columns
                                nc.sync.dma_start_transpose(
                                    CTX[t][:, ib * 512 + q * P : ib * 512 + (q + 1) * P],
                                    stg[:],
                                )
                if db == 5:
                    # one more chunk for blocks 0..3 (RMW into the parked
                    # partials), woven into db6's score stream
                    def wo_rmw(ib8, eb, f):
                        ps = psB.tile([P, 512], F32, tag="pj", name="pj")
                        nc.tensor.matmul(
                            ps[:],
                            CTX[f][:, ib8 * P : (ib8 + 1) * P],
                            WO[f][:, eb * 512 : (eb + 1) * 512],
                            start=True,
                            stop=True,
                        )
                        dst = QT[ib8][:, eb * 512 : (eb + 1) * 512]
                        nc.vector.tensor_add(dst, ps[:], dst)

                    fillers += [
                        lambda ib8=ib8, eb=eb: wo_rmw(ib8, eb, 5)
                        for ib8 in range(5)
                        for eb in range(2)
                    ]
                if db == 6:
                    fillers += [
                        lambda ib8=ib8, eb=eb: wo_rmw(ib8, eb, 6)
                        for ib8 in range(6)
                        for eb in range(2)
                    ]
                if db == 5:
                    fillers += [
                        lambda eb=eb: wo_part(5, eb, 6) for eb in range(2)
                    ]
                if db in (4, 6):
                    # partial Wo over the ctx tiles finished so far, woven as
                    # fillers between the next db's score tiles; partials
                    # (+ bias) park in QT tiles whose scores are done.
                    lo, hi = (0, 5) if db == 4 else (6, 7)
                    nf = 5 if db == 4 else 7

                    def wo_part(ib8, eb, nf):
                        ps = psB.tile([P, 512], F32, tag="pj", name="pj")
                        for f in range(nf):
                            nc.tensor.matmul(
                                ps[:],
                                CTX[f][:, ib8 * P : (ib8 + 1) * P],
                                WO[f][:, eb * 512 : (eb + 1) * 512],
                                start=(f == 0),
                                stop=(f == nf - 1),
                            )
                        nc.vector.tensor_add(
                            QT[ib8][:, eb * 512 : (eb + 1) * 512],
                            ps[:],
                            BIAS[:, eb * 512 : (eb + 1) * 512],
                        )

                    fillers += [
                        lambda ib8=ib8, eb=eb, nf=nf: wo_part(ib8, eb, nf)
                        for ib8 in range(lo, hi)
                        for eb in range(2)
                    ]

        if DEBUG:
            nc.sync.dma_start(dbg_ctx0, CTX[0][:])

        # ---------------- phase Wo, final pass ----------------------------
        # blocks ib8<=6 were partially accumulated into dead QT tiles during
        # the ACT-throttled late dbs; finish their remaining chunks here.
        for ib8 in range(NI // P):
            f0 = 7 if ib8 <= 6 else 0
            for eb in range(2):
                ps = psB.tile([P, 512], F32, tag="pj", name="pj")
                for f in range(f0, NCC):
                    nc.tensor.matmul(
                        ps[:],
                        CTX[f][:, ib8 * P : (ib8 + 1) * P],
                        WO[f][:, eb * 512 : (eb + 1) * 512],
                        start=(f == f0),
                        stop=(f == NCC - 1),
                    )
                ostage = osp.tile([P, 512], BF16, tag="os", name="os")
                prev = (
                    QT[ib8][:, eb * 512 : (eb + 1) * 512]
                    if ib8 <= 6
                    else BIAS[:, eb * 512 : (eb + 1) * 512]
                )
                nc.vector.tensor_add(ostage[:], ps[:], prev)
                nc.sync.dma_start(
                    out[ib8 * P : (ib8 + 1) * P, eb * 512 : (eb + 1) * 512],
                    ostage[:],
                )

    nc.compile()
    return nc


_NC = None


def _get_nc():
    global _NC
    if _NC is None:
        _NC = _build()
    return _NC


def _make_in_maps(x, Wq, Wk, Wv, Wo, bo):
    import ml_dtypes

    bf16 = ml_dtypes.bfloat16
    wqt = np.ascontiguousarray(Wq.T).astype(bf16)
    wkt = np.ascontiguousarray(Wk.T).astype(bf16)
    wvt = np.ascontiguousarray(Wv.T).astype(bf16)
    wot = np.ascontiguousarray(Wo.T).astype(bf16)
    bo2 = np.ascontiguousarray(bo.reshape(1, D)).astype(np.float32)
    in_maps = []
    for c in range(NCORES):
        b, s = c // 2, c % 2
        mine = x[b, s * NI : (s + 1) * NI, :].T
        other = x[b, (1 - s) * NI : (2 - s) * NI, :].T
        xtkv = np.ascontiguousarray(np.concatenate([mine, other], axis=1)).astype(bf16)
        in_maps.append(
            {"xtkv": xtkv, "wqt": wqt, "wkt": wkt, "wvt": wvt, "wot": wot, "bo": bo2}
        )
    return in_maps


def _run(x, Wq, Wk, Wv, Wo, bo, **spmd_kwargs):
    nc = _get_nc()
    in_maps = _make_in_maps(x, Wq, Wk, Wv, Wo, bo)
    res = run_bass_kernel_spmd(nc, in_maps, list(range(NCORES)), **spmd_kwargs)
    outs = [np.asarray(res.results[c]["out"]) for c in range(NCORES)]
    full = np.concatenate(outs, axis=0).reshape(4, 2048, D).astype(np.float32)
    return full, res


def kernel(x, Wq, Wk, Wv, Wo, bo):
    full, _ = _run(
        np.asarray(x), np.asarray(Wq), np.asarray(Wk), np.asarray(Wv),
        np.asarray(Wo), np.asarray(bo),
    )
    return full



# revision 76
# speedup vs baseline: 1.0727x; 1.0002x over previous
"""Multi-head attention (b=4, n=2048, d=1024, h=16, dh=64) on 8 TRN2 NeuronCores.

Sharding: batch x sequence-half per core (core c handles batch b=c//2, query
rows s=(c%2)*1024 .. +1024). Each core recomputes K/V for its whole batch
locally (no collectives), computes flash-style attention for its 1024 query
rows over all 16 heads, applies the output projection, and writes a disjoint
1024-row slice of the flattened output. Matmuls run in fp32r (TF32-like fast
fp32 mode); the attention AV product runs in bf16.

Host-side layout choices (free transposes/permutes in numpy):
  xtkv [d, 2048] = concat(x[b, my_half].T, x[b, other_half].T) -- the core's
      own query rows are ALWAYS columns 0:1024, so the same SPMD graph works
      on every core, and key order permutation is softmax-invariant.
  wqt/wkt/wvt/wot = W.T (contraction dim first), bo as [1, d].
"""

import sys

sys.path.insert(0, "/opt/trn_rl_repo")

from contextlib import ExitStack

import numpy as np

import concourse.bass as bass
import concourse.tile as tile
from concourse import bacc, mybir
from concourse.bass_utils import run_bass_kernel_spmd

F32 = mybir.dt.float32
F32R = mybir.dt.float32r
BF16 = mybir.dt.bfloat16
EXP = mybir.ActivationFunctionType.Exp

P = 128
D = 1024  # model dim
NI = 1024  # query rows per core
NJ = 2048  # key rows per core (full batch)
H = 16  # heads
DH = 64  # head dim
SCALE = DH**-0.5  # 0.125
NCORES = 8

NCC = D // P  # 8 contraction chunks
NDB = D // P  # 8 feature blocks


def _build():
    nc = bacc.Bacc("TRN2", target_bir_lowering=False, debug=False, num_devices=NCORES)

    xtkv = nc.dram_tensor("xtkv", [D, NJ], BF16, kind="ExternalInput").ap()
    wqt = nc.dram_tensor("wqt", [D, D], BF16, kind="ExternalInput").ap()
    wkt = nc.dram_tensor("wkt", [D, D], BF16, kind="ExternalInput").ap()
    wvt = nc.dram_tensor("wvt", [D, D], BF16, kind="ExternalInput").ap()
    wot = nc.dram_tensor("wot", [D, D], BF16, kind="ExternalInput").ap()
    bo = nc.dram_tensor("bo", [1, D], F32, kind="ExternalInput").ap()
    out = nc.dram_tensor("out", [NI, D], BF16, kind="ExternalOutput").ap()
    DEBUG = bool(__import__("os").environ.get("KERNEL_DEBUG"))
    if DEBUG:
        dbg_kt0 = nc.dram_tensor("dbg_kt0", [P, NJ], BF16, kind="ExternalOutput").ap()
        dbg_qt0 = nc.dram_tensor("dbg_qt0", [P, NI], BF16, kind="ExternalOutput").ap()
        dbg_v0 = nc.dram_tensor("dbg_v0", [P, H, DH + 1], BF16, kind="ExternalOutput").ap()
        dbg_ctx0 = nc.dram_tensor("dbg_ctx0", [P, NI], BF16, kind="ExternalOutput").ap()

    with tile.TileContext(nc) as tc, ExitStack() as octx:
        # kernel-wide PSUM pools: 4 + 2 + 2 = 8 banks
        psA = octx.enter_context(tc.tile_pool(name="psA", bufs=2, space="PSUM"))
        psB = octx.enter_context(tc.tile_pool(name="psB", bufs=2, space="PSUM"))
        psC = octx.enter_context(tc.tile_pool(name="psC", bufs=2, space="PSUM"))
        dramp = octx.enter_context(tc.tile_pool(name="dramp", bufs=4, space="DRAM"))

        kt_pool = octx.enter_context(tc.tile_pool(name="ktp", bufs=1))
        qt_pool = octx.enter_context(tc.tile_pool(name="qtp", bufs=1))
        v_pool = octx.enter_context(tc.tile_pool(name="vp", bufs=1))
        KT = [kt_pool.tile([P, NJ], BF16, tag=f"kt{i}", name=f"kt{i}") for i in range(NDB)]
        QT = [qt_pool.tile([P, NI], BF16, tag=f"qt{i}", name=f"qt{i}") for i in range(NDB)]
        vall = v_pool.tile([P, NJ // P, H, DH + 1], BF16, tag="vall", name="vall")
        V = [vall[:, j] for j in range(NJ // P)]

        # attention pools first: their SBUF must not alias the projection pools
        ctx_pool = octx.enter_context(tc.tile_pool(name="ctxp", bufs=1, side="right"))
        CTX = [ctx_pool.tile([P, NI], BF16, tag=f"ctx{t}", name=f"ctx{t}") for t in range(NDB)]
        esp = octx.enter_context(tc.tile_pool(name="es", bufs=11))
        recp = octx.enter_context(tc.tile_pool(name="rec", bufs=6))
        stp = octx.enter_context(tc.tile_pool(name="stg", bufs=10))

        # ---------------- phase Q (bf16); XQ reused by K; WK/XKB prefetched ----
        xqp = octx.enter_context(tc.tile_pool(name="xq", bufs=1))
        XQ = [xqp.tile([P, NI], BF16, tag=f"xq{c}", name=f"xq{c}") for c in range(NCC)]
        wkp = octx.enter_context(tc.tile_pool(name="wk", bufs=1))
        WK = [wkp.tile([P, D], BF16, tag=f"wk{c}", name=f"wk{c}") for c in range(NCC)]
        with tc.tile_pool(name="wq", bufs=1) as wqp:
            WQ = [wqp.tile([P, D], BF16, tag=f"wq{c}", name=f"wq{c}") for c in range(NCC)]
            # split the first loads so Q chain (db0, ib0) starts ~8us in
            # instead of waiting for all 4MB of interleaved XQ/WQ traffic
            for c in range(NCC):
                nc.sync.dma_start(WQ[c][:, 0 : 2 * P], wqt[c * P : (c + 1) * P, 0 : 2 * P])
            for c in range(NCC):
                nc.sync.dma_start(XQ[c][:, 0:512], xtkv[c * P : (c + 1) * P, 0:512])
            for c in range(NCC):
                nc.sync.dma_start(WQ[c][:, 2 * P : D], wqt[c * P : (c + 1) * P, 2 * P : D])
            for c in range(NCC):
                nc.sync.dma_start(XQ[c][:, 512:NI], xtkv[c * P : (c + 1) * P, 512:NI])
            for c in range(NCC):
                nc.sync.dma_start(WK[c][:], wkt[c * P : (c + 1) * P, :])
            for ib in range(NI // 512):
                for db in range(NDB):
                    ps = psB.tile([P, 512], F32, tag="pj", name="pj")
                    for c in range(NCC):
                        nc.tensor.matmul(
                            ps[:],
                            WQ[c][:, db * P : (db + 1) * P],
                            XQ[c][:, ib * 512 : (ib + 1) * 512],
                            start=(c == 0),
                            stop=(c == NCC - 1),
                        )
                    nc.vector.tensor_copy(QT[db][:, ib * 512 : (ib + 1) * 512], ps[:])

        # Wo pools open before the fused scope so WO/bias prefetch during it
        wop = octx.enter_context(tc.tile_pool(name="wo", bufs=1))
        bip = octx.enter_context(tc.tile_pool(name="bias", bufs=1))
        osp = octx.enter_context(tc.tile_pool(name="os", bufs=6))
        WO = [wop.tile([P, D], BF16, tag=f"wo{f}", name=f"wo{f}") for f in range(NCC)]
        for f in range(NCC):
            nc.sync.dma_start(WO[f][:], wot[f * P : (f + 1) * P, :])
        BIAS = bip.tile([P, D], F32, name="BIAS")
        nc.gpsimd.dma_start(BIAS[:], bo.to_broadcast([P, D]))

        # -------- fused phase K + attention: per db, project KT[db] then emit
        # the two heads (x2 ib blocks) that consume it ------------------------
        with (
            tc.tile_pool(name="xkb", bufs=1) as xkbp,
            tc.tile_pool(name="wvh", bufs=1) as wvhp,
        ):
            XKB = [xkbp.tile([P, NI], BF16, tag=f"xkb{c}", name=f"xkb{c}") for c in range(NCC)]
            XKA = XQ
            for c in range(NCC):
                nc.sync.dma_start(XKB[c][:], xtkv[c * P : (c + 1) * P, NI:NJ])
            for j in range(NJ // P):
                nc.vector.memset(V[j][:, :, DH : DH + 1], 1.0)
            fillers = []

            def v_halfpass_jg(vh, jg):
                """Project V head-half vh for key group jg (4 j-blocks); x is
                sliced straight out of the resident XKA/XKB tiles."""
                for j4 in range(4):
                    j = jg * 4 + j4
                    xh = XKA if j < 8 else XKB
                    jloc = j % 8
                    ps = psB.tile([P, 512], F32, tag="pj", name="pj")
                    for c in range(NCC):
                        nc.tensor.matmul(
                            ps[:],
                            xh[c][:, jloc * P : (jloc + 1) * P],
                            WVH[c][:],
                            start=(c == 0),
                            stop=(c == NCC - 1),
                        )
                    nc.vector.tensor_copy(
                        V[j][:, vh * 8 : (vh + 1) * 8, 0:DH],
                        ps[:].rearrange("p (h d) -> p h d", h=8),
                    )

            WVH = [wvhp.tile([P, 512], BF16, tag=f"wvh{c}", name=f"wvh{c}") for c in range(NCC)]
            # V head-half 0 (heads 0-7): needed from db=0
            for c in range(NCC):
                nc.sync.dma_start(WVH[c][:], wvt[c * P : (c + 1) * P, 0:512])
            for jg in range(NJ // 512):
                v_halfpass_jg(0, jg)

            for db in range(NDB):
                # V head-half 1 (heads 8-15): one key group per db in 1..4
                if db == 1:
                    WVH = [
                        wvhp.tile([P, 512], BF16, tag=f"wvh{c}", name=f"wvh{c}2")
                        for c in range(NCC)
                    ]
                    for c in range(NCC):
                        nc.sync.dma_start(WVH[c][:], wvt[c * P : (c + 1) * P, 512:1024])
                if 1 <= db <= 4:
                    v_halfpass_jg(1, db - 1)
                # K projection for this db
                for jb in range(NJ // 512):
                    half = XKA if jb < 2 else XKB
                    cslc = slice((jb % 2) * 512, (jb % 2) * 512 + 512)
                    ps = psB.tile([P, 512], F32, tag="pj", name="pj")
                    for c in range(NCC):
                        nc.tensor.matmul(
                            ps[:],
                            WK[c][:, db * P : (db + 1) * P],
                            half[c][:, cslc],
                            start=(c == 0),
                            stop=(c == NCC - 1),
                        )
                    nc.vector.tensor_copy(KT[db][:, jb * 512 : (jb + 1) * 512], ps[:])
                # the two heads living in KT[db], for both i-blocks
                t = db
                for ib in range(NI // 512):
                    islc = slice(ib * 512, (ib + 1) * 512)
                    stgs = {}
                    for hh in range(2):
                        h = 2 * db + hh
                        dp = hh * DH
                        es_list = []
                        for pr in range(NJ // 256):
                            sp = psA.tile([P, 1024], F32, tag="sp", name="sp")
                            for half2 in range(2):
                                j = pr * 2 + half2
                                nc.tensor.matmul(
                                    sp[:, half2 * 512 : (half2 + 1) * 512],
                                    KT[t][dp : dp + DH, j * P : (j + 1) * P],
                                    QT[t][dp : dp + DH, islc],
                                    start=True,
                                    stop=True,
                                )
                            es = esp.tile([P, 1024], BF16, tag="es", name="es")
                            nc.scalar.activation(es[:], sp[:], EXP, scale=SCALE)
                            es_list.append(es)
                            if fillers:
                                fillers.pop(0)()
                        # AV with es STATIONARY and V|1 MOVING: N=65 cycles per
                        # matmul instead of 512 -- output [i, d|sum] per i-128
                        # block, so the softmax divide is a native per-partition
                        # tensor_scalar, and a bf16 DMA-transpose restores the
                        # [f, i] layout Wo consumes.
                        for q in range(4):
                            ctp = psC.tile([P, DH + 1], F32, tag="ct", name="ct")
                            for j in range(NJ // P):
                                nc.tensor.matmul(
                                    ctp[:],
                                    es_list[j // 2][
                                        :,
                                        (j % 2) * 512 + q * P : (j % 2) * 512 + (q + 1) * P,
                                    ],
                                    V[j][:, h, :],
                                    start=(j == 0),
                                    stop=(j == NJ // P - 1),
                                )
                            rec = recp.tile([P, 1], F32, tag="rec", name="rec")
                            nc.vector.reciprocal(rec[:], ctp[:, DH : DH + 1])
                            if hh == 0:
                                stgs[q] = stp.tile([P, 2 * DH], BF16, tag="st", name="st")
                            stg = stgs[q]
                            nc.vector.tensor_scalar_mul(
                                stg[:, dp : dp + DH], ctp[:, 0:DH], rec[:]
                            )
                            if hh == 1:
                                # both heads of the pair staged: one 128-wide
                                # bf16 DMA-transpose fills CTX tile t's columns
                                nc.sync.dma_start_transpose(
                                    CTX[t][:, ib * 512 + q * P : ib * 512 + (q + 1) * P],
                                    stg[:],
                                )
                if db == 5:
                    # one more chunk for blocks 0..3 (RMW into the parked
                    # partials), woven into db6's score stream
                    def wo_rmw(ib8, eb, f):
                        ps = psB.tile([P, 512], F32, tag="pj", name="pj")
                        nc.tensor.matmul(
                            ps[:],
                            CTX[f][:, ib8 * P : (ib8 + 1) * P],
                            WO[f][:, eb * 512 : (eb + 1) * 512],
                            start=True,
                            stop=True,
                        )
                        dst = QT[ib8][:, eb * 512 : (eb + 1) * 512]
                        nc.vector.tensor_add(dst, ps[:], dst)

                    fillers += [
                        lambda ib8=ib8, eb=eb: wo_rmw(ib8, eb, 5)
                        for ib8 in range(5)
                        for eb in range(2)
                    ]
                if db == 6:
                    fillers += [
                        lambda ib8=ib8, eb=eb: wo_rmw(ib8, eb, 6)
                        for ib8 in range(6)
                        for eb in range(2)
                    ]
                if db == 5:
                    fillers += [
                        lambda eb=eb: wo_part(5, eb, 6) for eb in range(2)
                    ]
                if db in (4, 6):
                    # partial Wo over the ctx tiles finished so far, woven as
                    # fillers between the next db's score tiles; partials
                    # (+ bias) park in QT tiles whose scores are done.
                    lo, hi = (0, 5) if db == 4 else (6, 7)
                    nf = 5 if db == 4 else 7

                    def wo_part(ib8, eb, nf):
                        ps = psB.tile([P, 512], F32, tag="pj", name="pj")
                        for f in range(nf):
                            nc.tensor.matmul(
                                ps[:],
                                CTX[f][:, ib8 * P : (ib8 + 1) * P],
                                WO[f][:, eb * 512 : (eb + 1) * 512],
                                start=(f == 0),
                                stop=(f == nf - 1),
                            )
                        nc.vector.tensor_add(
                            QT[ib8][:, eb * 512 : (eb + 1) * 512],
                            ps[:],
                            BIAS[:, eb * 512 : (eb + 1) * 512],
                        )

                    fillers += [
                        lambda ib8=ib8, eb=eb, nf=nf: wo_part(ib8, eb, nf)
                        for ib8 in range(lo, hi)
                        for eb in range(2)
                    ]
                if db == 6:
                    # ib8=7's partial parks in XKB[0], dead once db7's K
                    # projection (emitted before these fillers pop) consumed it
                    def wo_part7(eb):
                        ps = psB.tile([P, 512], F32, tag="pj", name="pj")
                        for f in range(7):
                            nc.tensor.matmul(
                                ps[:],
                                CTX[f][:, 7 * P : 8 * P],
                                WO[f][:, eb * 512 : (eb + 1) * 512],
                                start=(f == 0),
                                stop=(f == 6),
                            )
                        nc.vector.tensor_add(
                            XKB[0][:, eb * 512 : (eb + 1) * 512],
                            ps[:],
                            BIAS[:, eb * 512 : (eb + 1) * 512],
                        )

                    fillers += [lambda eb=eb: wo_part7(eb) for eb in range(2)]

            # ------------- phase Wo, final pass (inside xkb scope) --------
            # every block was partially accumulated into dead QT/XKB tiles
            # during the ACT-throttled late dbs; one chunk left per block.
            for ib8 in range(NI // P):
                for eb in range(2):
                    ps = psB.tile([P, 512], F32, tag="pj", name="pj")
                    nc.tensor.matmul(
                        ps[:],
                        CTX[7][:, ib8 * P : (ib8 + 1) * P],
                        WO[7][:, eb * 512 : (eb + 1) * 512],
                        start=True,
                        stop=True,
                    )
                    ostage = osp.tile([P, 512], BF16, tag="os", name="os")
                    prev = (
                        QT[ib8][:, eb * 512 : (eb + 1) * 512]
                        if ib8 <= 6
                        else XKB[0][:, eb * 512 : (eb + 1) * 512]
                    )
                    nc.vector.tensor_add(ostage[:], ps[:], prev)
                    nc.sync.dma_start(
                        out[ib8 * P : (ib8 + 1) * P, eb * 512 : (eb + 1) * 512],
                        ostage[:],
                    )

        if DEBUG:
            nc.sync.dma_start(dbg_ctx0, CTX[0][:])

    nc.compile()
    return nc


_NC = None


def _get_nc():
    global _NC
    if _NC is None:
        _NC = _build()
    return _NC


def _make_in_maps(x, Wq, Wk, Wv, Wo, bo):
    import ml_dtypes

    bf16 = ml_dtypes.bfloat16
    wqt = np.ascontiguousarray(Wq.T).astype(bf16)
    wkt = np.ascontiguousarray(Wk.T).astype(bf16)
    wvt = np.ascontiguousarray(Wv.T).astype(bf16)
    wot = np.ascontiguousarray(Wo.T).astype(bf16)
    bo2 = np.ascontiguousarray(bo.reshape(1, D)).astype(np.float32)
    in_maps = []
    for c in range(NCORES):
        b, s = c // 2, c % 2
        mine = x[b, s * NI : (s + 1) * NI, :].T
        other = x[b, (1 - s) * NI : (2 - s) * NI, :].T
        xtkv = np.ascontiguousarray(np.concatenate([mine, other], axis=1)).astype(bf16)
        in_maps.append(
            {"xtkv": xtkv, "wqt": wqt, "wkt": wkt, "wvt": wvt, "wot": wot, "bo": bo2}
        )
    return in_maps


def _run(x, Wq, Wk, Wv, Wo, bo, **spmd_kwargs):
    nc = _get_nc()
    in_maps = _make_in_maps(x, Wq, Wk, Wv, Wo, bo)
    res = run_bass_kernel_spmd(nc, in_maps, list(range(NCORES)), **spmd_kwargs)
    outs = [np.asarray(res.results[c]["out"]) for c in range(NCORES)]
    full = np.concatenate(outs, axis=0).reshape(4, 2048, D).astype(np.float32)
    return full, res


def kernel(x, Wq, Wk, Wv, Wo, bo):
    full, _ = _run(
        np.asarray(x), np.asarray(Wq), np.asarray(Wk), np.asarray(Wv),
        np.asarray(Wo), np.asarray(bo),
    )
    return full

